# revision 29
# baseline (speedup 1.0000x reference)
"""AChebyKANLinear forward on 8 TRN2 NeuronCores (data-parallel over batch).

y = silu(x) @ W_base^T + einsum('bid,iod->bo', cos(n_d * arccos(tanh x)), gated_coeffs)

Key identities used:
  cos(n*arccos(c)) = T_n(c)  (Chebyshev), c = tanh(x)
  -> no trig needed on device. Device computes 13 "columns" per feature:
     silu(x), and 12 cheap polynomials of c whose exact Chebyshev-basis
     expansion is tracked symbolically on the host; the host solves a small
     linear system to fold the change of basis into the matmul weights.
  Even T_2m come from ACT Square ops (T_2m+1 = 2*T_m^2), odd ones from single
  fused DVE scalar_tensor_tensor ops. All columns bf16; one big
  [4096, 3328] @ [3328, 256] GEMM per core on TensorE (fp32 PSUM accum).

Top-k routing over the 8 logits is computed on the host (it is 8 numbers);
the 4 selected high degrees are baked into the compiled graph.
"""

import numpy as np
import ml_dtypes
from contextlib import ExitStack

import concourse.bass as bass
import concourse.tile as tile
from concourse import bacc, mybir
from concourse.bass_utils import run_bass_kernel_spmd

BF16 = ml_dtypes.bfloat16

N_CORES = 8
BATCH, I_DIM, O_DIM = 32768, 256, 256
B_LOC = BATCH // N_CORES          # 4096
# graduated batch chunks: small first chunks shorten the pipeline fill before
# TensorE has all 13 columns of chunk 0; steady state runs at 512.
CHUNK_SIZES = [256, 256, 512, 1024, 1024, 1024]
assert sum(CHUNK_SIZES) == B_LOC
BC_MAX = max(CHUNK_SIZES)
DEGREE = 16
BASE_DEGREES = 8
TOPK = 4

SQ2 = float(np.sqrt(2.0))

A = mybir.ActivationFunctionType
ALU = mybir.AluOpType
F32 = mybir.dt.float32
DBF16 = mybir.dt.bfloat16


# ---------------- symbolic Chebyshev algebra (host, exact) ----------------

def _chmul(a, b):
    out = np.zeros(40)
    nz_a = np.nonzero(a)[0]
    nz_b = np.nonzero(b)[0]
    for i in nz_a:
        for j in nz_b:
            p = a[i] * b[j] * 0.5
            out[i + j] += p
            out[abs(i - j)] += p
    return out


def _e(n):
    v = np.zeros(40)
    v[n] = 1.0
    return v


def _recipe(S):
    """Build the per-chunk op recipe and each column's Chebyshev expansion.

    Returns (ops, colvec) where ops is a list of
      ('act', name, src, func, scale, bias) or
      ('stt', name, in0, scalar, op0, in1, op1) or
      ('tt',  name, in0, in1, op)
    and colvec maps tile name -> length-40 Chebyshev coefficient vector.
    """
    ops = []
    vec = {}

    def act(name, src, func, scale=1.0, bias=0.0):
        ops.append(("act", name, src, func, float(scale), float(bias)))
        if func == A.Square:
            aff = vec[src] * scale
            aff[0] += bias
            vec[name] = _chmul(aff, aff)
        elif func == A.Tanh:
            vec[name] = _e(1)
        else:  # Silu: not a Chebyshev column
            vec[name] = None

    def stt(name, in0, scalar, op0, in1, op1):
        ops.append(("stt", name, in0, float(scalar), op0, in1, op1))
        a = vec[in0].copy()
        if op0 == ALU.add:
            a[0] += scalar
        elif op0 == ALU.mult:
            a = a * scalar
        else:
            raise ValueError(op0)
        b = vec[in1]
        if op1 == ALU.mult:
            vec[name] = _chmul(a, b)
        elif op1 == ALU.subtract:
            vec[name] = a - b
        elif op1 == ALU.add:
            vec[name] = a + b
        else:
            raise ValueError(op1)

    def tt(name, in0, in1, op):
        ops.append(("tt", name, in0, in1, op))
        if op == ALU.subtract:
            vec[name] = vec[in0] - vec[in1]
        elif op == ALU.add:
            vec[name] = vec[in0] + vec[in1]
        elif op == ALU.mult:
            vec[name] = _chmul(vec[in0], vec[in1])
        else:
            raise ValueError(op)

    act("silu", "x", A.Silu)
    act("c1", "x", A.Tanh)
    act("c2", "c1", A.Square, SQ2)            # T2 + 1
    act("c4", "c2", A.Square, SQ2, -SQ2)      # T4 + 1
    act("c8", "c4", A.Square, SQ2, -SQ2)      # T8 + 1
    stt("c3", "c2", -1.5, ALU.add, "c1", ALU.mult)   # (c2-1.5)*c1 = T3/2
    stt("c5", "c4", -1.0, ALU.add, "c1", ALU.mult)   # T4*T1 = (T5+T3)/2
    stt("c6", "c3", 2.0, ALU.mult, "c3", ALU.mult)   # 2*c3^2 = (T6+1)/2  (DVE)
    stt("c7", "c4", -1.0, ALU.add, "c3", ALU.mult)   # T4*T3/2 = (T7+T1)/4
    for n in S:
        if n == 9:
            stt("c9", "c8", -1.0, ALU.add, "c1", ALU.mult)    # T8*T1
        elif n == 10:
            act("c10", "c5", A.Square, SQ2)                    # 2*c5^2
        elif n == 11:
            stt("c11", "c8", -1.0, ALU.add, "c3", ALU.mult)   # T8*T3/2
        elif n == 12:
            act("c12", "c6", A.Square, 2.0 * SQ2, -SQ2)        # 2*(2*c6-1)^2 = T12+1
        elif n == 13:
            tt("d53", "c5", "c3", ALU.subtract)                # T5/2
            stt("c13", "c8", -1.0, ALU.add, "d53", ALU.mult)  # T8*T5/2
        elif n == 14:
            stt("c14", "c8", -1.0, ALU.add, "c6", ALU.mult)   # T8*(T6+1)/2
        elif n == 15:
            stt("t7p", "c7", 4.0, ALU.mult, "c1", ALU.subtract)  # T7
            stt("c15", "c8", -1.0, ALU.add, "t7p", ALU.mult)     # T8*T7
        elif n == 16:
            act("c16", "c8", A.Square, SQ2, -SQ2)              # T16+1
        else:
            raise ValueError(n)
    return ops, vec


def _solve_basis(S, low_degrees):
    """Solve for X s.t. sum_col X[col,n]*colvec[col] == e_n for each needed n.

    Columns: 'bias' (the constant 1) + the 12 device Chebyshev columns.
    Returns (ops, matmul_cols, X) with X keyed [col][n].
    """
    ops, vec = _recipe(S)
    cheb_cols = ["c1", "c2", "c3", "c4", "c5", "c6", "c7", "c8"] + [f"c{n}" for n in S]
    needed = sorted(set(int(n) for n in low_degrees) | set(S))
    Amat = np.zeros((40, 1 + len(cheb_cols)))
    Amat[0, 0] = 1.0  # bias column = T_0
    for j, cn in enumerate(cheb_cols):
        Amat[:, 1 + j] = vec[cn]
    X = {}
    for n in needed:
        sol, res, rank, _ = np.linalg.lstsq(Amat, _e(n), rcond=None)
        err = np.abs(Amat @ sol - _e(n)).max()
        assert err < 1e-9, f"basis solve failed for degree {n}: {err}"
        X[n] = sol  # [1+len(cheb_cols)]
    return ops, cheb_cols, X


# ---------------- device graph ----------------

# fp8 "leaf" columns: columns no chain op reads can be written float8_e4m3
# directly and contracted with DoubleRow matmuls (2 k-rows per instruction).
# All weights are scaled by W_SCALE on host (so fp8 weights avoid subnormals);
# the PSUM is descaled during the bias-add evacuation.
# Measured on HW: DoubleRow groups are 1.4x faster in an isolated microbench,
# but in this kernel the 256-col non-FWL LDWEIGHTS exposure cancels the gain
# (A/B medians 94us fp8 vs 91us bf16) while costing rel-err 8e-3 vs 4.2e-3.
# Kept implemented but disabled.
FP8_LEAVES = False
R_BUFS = 2
T_BUFS = 2
W_SCALE = 4096.0
DF8 = mybir.dt.float8e4
F8NP = mybir.dt.np(mybir.dt.float8e4)


def _leaf_cols(ops, cheb_cols):
    if not FP8_LEAVES:
        return []
    read = set()
    for op in ops:
        if op[0] == "act":
            read.add(op[2])
        elif op[0] == "stt":
            read.add(op[2])
            read.add(op[5])
        else:
            read.add(op[2])
            read.add(op[3])
    return [c for c in cheb_cols if c not in read]


def _build_nc(S, niter=1, ablate=None):
    # ablate: None (normal), "producers" (memset columns once; PE/DMA path only)
    ops, cheb_cols, _ = _solve_basis(S, range(BASE_DEGREES + 1))
    leaves = _leaf_cols(ops, cheb_cols)
    bf_blocks = ["silu"] + [c for c in cheb_cols if c not in leaves]
    n_kk = 2 * len(bf_blocks)              # bf16 k-tiles of 128
    n8 = len(leaves)                       # fp8 DoubleRow blocks (256 k-rows each)

    nc = bacc.Bacc("TRN2", target_bir_lowering=False, debug=False,
                   num_devices=N_CORES)
    # register const APs for the activation bias values we use (only 0.0/1.0
    # are pre-registered); mirrors Bass.__init__'s register_const_ap.
    bias_consts = sorted({op[5] for op in ops if op[0] == "act"} - {0.0})
    for v in bias_consts:
        t_c = nc.alloc_sbuf_tensor(f"const-f32-{v}", [128, 1], F32)
        nc.gpsimd.memset(t_c.ap(), v)
        nc.const_aps.aps[(F32, v)] = t_c.ap()
    if bias_consts:
        nc.all_engine_barrier()
    x_d = nc.dram_tensor("xt", [128, 2 * B_LOC], F32, kind="ExternalInput").ap()
    w_d = nc.dram_tensor("w", [128, n_kk * O_DIM], DBF16, kind="ExternalInput").ap()
    if n8:
        w8_d = nc.dram_tensor("w8", [128, n8 * 2 * O_DIM], DF8,
                              kind="ExternalInput").ap()
    b_d = nc.dram_tensor("bias", [O_DIM, 1], F32, kind="ExternalInput").ap()
    o_d = nc.dram_tensor("out", [O_DIM, B_LOC], F32, kind="ExternalOutput").ap()

    with tile.TileContext(nc) as tc, ExitStack() as ctx:
        cpool = ctx.enter_context(tc.tile_pool(name="const", bufs=1))
        xpool = ctx.enter_context(tc.tile_pool(name="x", bufs=3))
        rpool = ctx.enter_context(tc.tile_pool(name="r", bufs=R_BUFS))
        tpool = ctx.enter_context(tc.tile_pool(name="tmp", bufs=T_BUFS))
        opool = ctx.enter_context(tc.tile_pool(name="o", bufs=8))
        pspool = ctx.enter_context(tc.tile_pool(name="ps", bufs=8, space="PSUM"))

        wt = cpool.tile([128, n_kk * O_DIM], DBF16)
        nc.sync.dma_start(wt[:], w_d[:])
        if n8:
            w8t = cpool.tile([128, n8 * 2 * O_DIM], DF8)
            nc.sync.dma_start(w8t[:], w8_d[:])
            w8v = w8t[:].rearrange("p (b r o) -> p b r o", r=2, o=O_DIM)
        bt = []
        for m in range(2):
            b_tile = cpool.tile([128, 1], F32, tag=f"bias{m}")
            nc.sync.dma_start(b_tile[:], b_d[m * 128:(m + 1) * 128, :])
            bt.append(b_tile)

        chunks = []
        off = 0
        for bc in CHUNK_SIZES:
            chunks.append((off, bc))
            off += bc
        const_tiles = None
        if ablate == "producers":
            const_tiles = {}
            for nm in (["silu"] + cheb_cols):
                ct = cpool.tile([128, 2 * BC_MAX],
                                DF8 if nm in leaves else DBF16,
                                tag=f"ab_{nm}", name=f"ab_{nm}")
                nc.vector.memset(ct[:], 0.5)
                const_tiles[nm] = ct
        for it in range(niter):
            for ci, (off, bc) in enumerate(chunks):
                cc = f"{it}_{ci}"
                xt = xpool.tile([128, 2 * bc], F32, tag="xt", name=f"xt{cc}")
                nc.sync.dma_start(xt[:], x_d[:, 2 * off: 2 * (off + bc)])

                tiles = {"x": xt}
                if ablate == "producers":
                    tiles.update(const_tiles)
                else:
                    for op in ops:
                        kind, name = op[0], op[1]
                        is_col = (name in bf_blocks) or (name in leaves)
                        pool = rpool if is_col else tpool
                        dt_col = DF8 if name in leaves else DBF16
                        t = pool.tile([128, 2 * bc], dt_col, tag=name,
                                      name=f"{name}_{cc}")
                        if kind == "act":
                            _, _, src, func, scale, bias_v = op
                            nc.scalar.activation(t[:], tiles[src][:], func,
                                                 bias=bias_v, scale=scale)
                        elif kind == "stt":
                            _, _, in0, scalar, op0, in1, op1 = op
                            nc.vector.scalar_tensor_tensor(t[:], tiles[in0][:], scalar,
                                                           tiles[in1][:], op0, op1)
                        else:  # tt
                            _, _, in0, in1, alu = op
                            nc.vector.tensor_tensor(t[:], tiles[in0][:],
                                                    tiles[in1][:], alu)
                        tiles[name] = t

                nsubs = [(s, min(512, bc - s)) for s in range(0, bc, 512)]
                n_mm = n_kk + n8
                for m in range(2):
                    for so, sn in nsubs:
                        ps = pspool.tile([128, sn], F32, tag="ps",
                                         name=f"ps{cc}_{m}_{so}")
                        mi = 0
                        for kk in range(n_kk):
                            j, h = kk // 2, kk % 2
                            rt = tiles[bf_blocks[j]]
                            nc.tensor.matmul(
                                ps[:],
                                wt[:, kk * O_DIM + m * 128: kk * O_DIM + (m + 1) * 128],
                                rt[:, h * bc + so: h * bc + so + sn],
                                start=(mi == 0), stop=(mi == n_mm - 1),
                            )
                            mi += 1
                        for b8, lf in enumerate(leaves):
                            rt = tiles[lf]
                            rhs = rt[:].rearrange("p (r n) -> p r n", r=2)[:, :, so:so + sn]
                            nc.tensor.matmul(
                                ps[:],
                                w8v[:, b8, :, m * 128:(m + 1) * 128],
                                rhs,
                                start=(mi == 0), stop=(mi == n_mm - 1),
                                perf_mode=mybir.MatmulPerfMode.DoubleRow,
                            )
                            mi += 1
                        ot = opool.tile([128, sn], F32, tag="ot",
                                        name=f"ot{cc}_{m}_{so}")
                        nc.vector.tensor_scalar(ot[:], ps[:], 1.0 / W_SCALE,
                                                bt[m][:], ALU.mult, ALU.add)
                        nc.sync.dma_start(
                            o_d[m * 128:(m + 1) * 128, off + so: off + so + sn],
                            ot[:])

    nc.compile()
    return nc


_NC_CACHE = {}


def _get_nc(S, niter=1):
    key = (tuple(S), niter)
    if key not in _NC_CACHE:
        _NC_CACHE[key] = _build_nc(S, niter)
    return _NC_CACHE[key]


# ---------------- host wrapper ----------------

def _prepare(x, logits, cheby_coeffs, base_weight, gating_weights, arange):
    x = np.asarray(x, dtype=np.float32)
    logits = np.asarray(logits, dtype=np.float32)
    cheby_coeffs = np.asarray(cheby_coeffs, dtype=np.float32)
    base_weight = np.asarray(base_weight, dtype=np.float32)
    gating_weights = np.asarray(gating_weights, dtype=np.float32)
    arange = np.asarray(arange)

    # top-k routing (host; 8 numbers). Matches jax.lax.top_k ordering.
    order = np.argsort(-logits, kind="stable")[:TOPK]
    topk_vals = 1.0 / (1.0 + np.exp(-logits[order].astype(np.float64)))
    gate = gating_weights.astype(np.float64).copy()
    sel = order + BASE_DEGREES + 1
    gate[sel] = topk_vals
    S = sorted(int(v) for v in sel)

    low = sorted(int(v) for v in arange)   # normally [0..8]
    ops, cheb_cols, X = _solve_basis(S, low)
    _, vec = _recipe(S)
    leaves = _leaf_cols(ops, cheb_cols)
    bf_cheb = [c for c in cheb_cols if c not in leaves]

    # true (f64) weight blocks per column via the basis solve
    G = {n: gate[n] * cheby_coeffs[:, :, n].astype(np.float64) for n in set(low) | set(S)}
    Wtrue = {"silu": base_weight.T.astype(np.float64)}
    bias = np.zeros(O_DIM, dtype=np.float64)
    for j, cn in enumerate(cheb_cols):
        Wb = np.zeros((I_DIM, O_DIM), dtype=np.float64)
        for n, sol in X.items():
            coef = sol[1 + j]
            if coef != 0.0 and n in G:
                Wb += coef * G[n]
        Wtrue[cn] = Wb
    for n, sol in X.items():
        if sol[0] != 0.0 and n in G:
            bias += sol[0] * G[n].sum(axis=0)

    # fp8-quantize leaf blocks (scaled); error-feedback: fold the recoverable
    # Chebyshev content of the quantization residual into the bf16 blocks.
    W8q = {}
    Wadd = {cn: 0.0 for cn in bf_cheb}
    if leaves:
        Alow = np.zeros((40, 1 + len(bf_cheb)))
        Alow[0, 0] = 1.0
        for j, cn in enumerate(bf_cheb):
            Alow[:, 1 + j] = vec[cn]
        comp = {}
        for cn in leaves:
            q = (Wtrue[cn] * W_SCALE).astype(np.float32).astype(F8NP)
            W8q[cn] = q
            dW = Wtrue[cn] - q.astype(np.float64) / W_SCALE
            v = vec[cn]
            for n in np.nonzero(np.abs(v) > 1e-9)[0]:
                if n == 0:
                    bias += v[0] * dW.sum(axis=0)
                else:
                    comp[int(n)] = comp.get(int(n), 0.0) + v[n] * dW
        for n, V in comp.items():
            sol, *_ = np.linalg.lstsq(Alow, _e(n), rcond=None)
            if np.abs(Alow @ sol - _e(n)).max() > 1e-9:
                continue  # degree only carried by an fp8 column; leave as is
            bias += sol[0] * V.sum(axis=0)
            for j, cn in enumerate(bf_cheb):
                if sol[1 + j] != 0.0:
                    Wadd[cn] = Wadd[cn] + sol[1 + j] * V

    # device layouts (all weights scaled by W_SCALE; evac descales)
    bf_blocks = ["silu"] + bf_cheb
    Wsb = np.empty((128, 2 * len(bf_blocks), O_DIM), dtype=np.float32)
    for j, cn in enumerate(bf_blocks):
        Wf = ((Wtrue[cn] + (Wadd.get(cn, 0.0))) * W_SCALE).astype(np.float32)
        Wsb[:, 2 * j + 0, :] = Wf[0:128, :]
        Wsb[:, 2 * j + 1, :] = Wf[128:256, :]
    w_np = Wsb.reshape(128, 2 * len(bf_blocks) * O_DIM).astype(BF16)
    w8_np = None
    if leaves:
        W8sb = np.empty((128, 2 * len(leaves), O_DIM), dtype=F8NP)
        for b8, cn in enumerate(leaves):
            W8sb[:, 2 * b8 + 0, :] = W8q[cn][0:128, :]
            W8sb[:, 2 * b8 + 1, :] = W8q[cn][128:256, :]
        w8_np = W8sb.reshape(128, 2 * len(leaves) * O_DIM)
    bias_np = bias.astype(np.float32).reshape(O_DIM, 1)
    return S, w_np, w8_np, bias_np


def _make_xt(xl):
    """xt[p, 2*off + h*bc + bb] = xl[off+bb, 128*h+p] for each chunk (off, bc)."""
    out = np.empty((128, 2 * B_LOC), dtype=np.float32)
    off = 0
    for bc in CHUNK_SIZES:
        blk = xl[off:off + bc, :].reshape(bc, 2, 128).transpose(2, 1, 0)
        out[:, 2 * off: 2 * (off + bc)] = blk.reshape(128, 2 * bc)
        off += bc
    return out


def _make_in_maps(x, w_np, w8_np, bias_np):
    in_maps = []
    for c in range(N_CORES):
        m = {"xt": _make_xt(x[c * B_LOC:(c + 1) * B_LOC, :]),
             "w": w_np, "bias": bias_np}
        if w8_np is not None:
            m["w8"] = w8_np
        in_maps.append(m)
    return in_maps


def kernel(x, t, logits, cheby_coeffs, base_weight, gating_weights, arange):
    x = np.asarray(x, dtype=np.float32)
    S, w_np, w8_np, bias_np = _prepare(x, logits, cheby_coeffs, base_weight,
                                       gating_weights, arange)
    nc = _get_nc(S)
    in_maps = _make_in_maps(x, w_np, w8_np, bias_np)
    res = run_bass_kernel_spmd(nc, in_maps, core_ids=list(range(N_CORES)))
    y = np.empty((BATCH, O_DIM), dtype=np.float32)
    for c in range(N_CORES):
        y[c * B_LOC:(c + 1) * B_LOC, :] = res.results[c]["out"].T
    return y


# revision 31
# speedup vs baseline: 1.0904x; 1.0904x over previous
"""AChebyKANLinear forward on 8 TRN2 NeuronCores (data-parallel over batch).

y = silu(x) @ W_base^T + einsum('bid,iod->bo', cos(n_d * arccos(tanh x)), gated_coeffs)

Key identities used:
  cos(n*arccos(c)) = T_n(c)  (Chebyshev), c = tanh(x)
  -> no trig needed on device. Device computes 13 "columns" per feature:
     silu(x), and 12 cheap polynomials of c whose exact Chebyshev-basis
     expansion is tracked symbolically on the host; the host solves a small
     linear system to fold the change of basis into the matmul weights.
  Even T_2m come from ACT Square ops (T_2m+1 = 2*T_m^2), odd ones from single
  fused DVE scalar_tensor_tensor ops. All columns bf16; one big
  [4096, 3328] @ [3328, 256] GEMM per core on TensorE (fp32 PSUM accum).

Top-k routing over the 8 logits is computed on the host (it is 8 numbers);
the 4 selected high degrees are baked into the compiled graph.
"""

import numpy as np
import ml_dtypes
from contextlib import ExitStack

import concourse.bass as bass
import concourse.tile as tile
from concourse import bacc, mybir
from concourse.bass_utils import run_bass_kernel_spmd

BF16 = ml_dtypes.bfloat16

N_CORES = 8
BATCH, I_DIM, O_DIM = 32768, 256, 256
B_LOC = BATCH // N_CORES          # 4096
# graduated batch chunks: small first chunks shorten the pipeline fill before
# TensorE has all 13 columns of chunk 0; steady state runs at 512.
CHUNK_SIZES = [256, 256, 512, 1024, 1024, 1024]
assert sum(CHUNK_SIZES) == B_LOC
BC_MAX = max(CHUNK_SIZES)
DEGREE = 16
BASE_DEGREES = 8
TOPK = 4

SQ2 = float(np.sqrt(2.0))

A = mybir.ActivationFunctionType
ALU = mybir.AluOpType
F32 = mybir.dt.float32
DBF16 = mybir.dt.bfloat16


# ---------------- symbolic Chebyshev algebra (host, exact) ----------------

def _chmul(a, b):
    out = np.zeros(40)
    nz_a = np.nonzero(a)[0]
    nz_b = np.nonzero(b)[0]
    for i in nz_a:
        for j in nz_b:
            p = a[i] * b[j] * 0.5
            out[i + j] += p
            out[abs(i - j)] += p
    return out


def _e(n):
    v = np.zeros(40)
    v[n] = 1.0
    return v


def _recipe(S):
    """Build the per-chunk op recipe and each column's Chebyshev expansion.

    Returns (ops, colvec) where ops is a list of
      ('act', name, src, func, scale, bias) or
      ('stt', name, in0, scalar, op0, in1, op1) or
      ('tt',  name, in0, in1, op)
    and colvec maps tile name -> length-40 Chebyshev coefficient vector.
    """
    ops = []
    vec = {}

    def act(name, src, func, scale=1.0, bias=0.0):
        ops.append(("act", name, src, func, float(scale), float(bias)))
        if func == A.Square:
            aff = vec[src] * scale
            aff[0] += bias
            vec[name] = _chmul(aff, aff)
        elif func == A.Tanh:
            vec[name] = _e(1)
        else:  # Silu: not a Chebyshev column
            vec[name] = None

    def stt(name, in0, scalar, op0, in1, op1):
        ops.append(("stt", name, in0, float(scalar), op0, in1, op1))
        a = vec[in0].copy()
        if op0 == ALU.add:
            a[0] += scalar
        elif op0 == ALU.mult:
            a = a * scalar
        else:
            raise ValueError(op0)
        b = vec[in1]
        if op1 == ALU.mult:
            vec[name] = _chmul(a, b)
        elif op1 == ALU.subtract:
            vec[name] = a - b
        elif op1 == ALU.add:
            vec[name] = a + b
        else:
            raise ValueError(op1)

    def tt(name, in0, in1, op):
        ops.append(("tt", name, in0, in1, op))
        if op == ALU.subtract:
            vec[name] = vec[in0] - vec[in1]
        elif op == ALU.add:
            vec[name] = vec[in0] + vec[in1]
        elif op == ALU.mult:
            vec[name] = _chmul(vec[in0], vec[in1])
        else:
            raise ValueError(op)

    # c1 first: it unblocks the whole Square chain and every DVE op; silu is
    # only consumed by the last K-blocks, so it is produced late.
    act("c1", "x", A.Tanh)
    act("c2", "c1", A.Square, SQ2)            # T2 + 1
    stt("c3", "c2", -1.5, ALU.add, "c1", ALU.mult)   # (c2-1.5)*c1 = T3/2
    act("c4", "c2", A.Square, SQ2, -SQ2)      # T4 + 1
    stt("c5", "c4", -1.0, ALU.add, "c1", ALU.mult)   # T4*T1 = (T5+T3)/2
    stt("c6", "c3", 2.0, ALU.mult, "c3", ALU.mult)   # 2*c3^2 = (T6+1)/2  (DVE)
    stt("c7", "c4", -1.0, ALU.add, "c3", ALU.mult)   # T4*T3/2 = (T7+T1)/4
    act("c8", "c4", A.Square, SQ2, -SQ2)      # T8 + 1
    act("silu", "x", A.Silu)
    for n in S:
        if n == 9:
            stt("c9", "c8", -1.0, ALU.add, "c1", ALU.mult)    # T8*T1
        elif n == 10:
            act("c10", "c5", A.Square, SQ2)                    # 2*c5^2
        elif n == 11:
            stt("c11", "c8", -1.0, ALU.add, "c3", ALU.mult)   # T8*T3/2
        elif n == 12:
            act("c12", "c6", A.Square, 2.0 * SQ2, -SQ2)        # 2*(2*c6-1)^2 = T12+1
        elif n == 13:
            tt("d53", "c5", "c3", ALU.subtract)                # T5/2
            stt("c13", "c8", -1.0, ALU.add, "d53", ALU.mult)  # T8*T5/2
        elif n == 14:
            stt("c14", "c8", -1.0, ALU.add, "c6", ALU.mult)   # T8*(T6+1)/2
        elif n == 15:
            stt("t7p", "c7", 4.0, ALU.mult, "c1", ALU.subtract)  # T7
            stt("c15", "c8", -1.0, ALU.add, "t7p", ALU.mult)     # T8*T7
        elif n == 16:
            act("c16", "c8", A.Square, SQ2, -SQ2)              # T16+1
        else:
            raise ValueError(n)
    return ops, vec


def _solve_basis(S, low_degrees):
    """Solve for X s.t. sum_col X[col,n]*colvec[col] == e_n for each needed n.

    Columns: 'bias' (the constant 1) + the 12 device Chebyshev columns.
    Returns (ops, matmul_cols, X) with X keyed [col][n].
    """
    ops, vec = _recipe(S)
    cheb_cols = ["c1", "c2", "c3", "c4", "c5", "c6", "c7", "c8"] + [f"c{n}" for n in S]
    needed = sorted(set(int(n) for n in low_degrees) | set(S))
    Amat = np.zeros((40, 1 + len(cheb_cols)))
    Amat[0, 0] = 1.0  # bias column = T_0
    for j, cn in enumerate(cheb_cols):
        Amat[:, 1 + j] = vec[cn]
    X = {}
    for n in needed:
        sol, res, rank, _ = np.linalg.lstsq(Amat, _e(n), rcond=None)
        err = np.abs(Amat @ sol - _e(n)).max()
        assert err < 1e-9, f"basis solve failed for degree {n}: {err}"
        X[n] = sol  # [1+len(cheb_cols)]
    return ops, cheb_cols, X


# ---------------- device graph ----------------

# fp8 "leaf" columns: columns no chain op reads can be written float8_e4m3
# directly and contracted with DoubleRow matmuls (2 k-rows per instruction).
# All weights are scaled by W_SCALE on host (so fp8 weights avoid subnormals);
# the PSUM is descaled during the bias-add evacuation.
# Measured on HW: DoubleRow groups are 1.4x faster in an isolated microbench,
# but in this kernel the 256-col non-FWL LDWEIGHTS exposure cancels the gain
# (A/B medians 94us fp8 vs 91us bf16) while costing rel-err 8e-3 vs 4.2e-3.
# Kept implemented but disabled.
FP8_LEAVES = False
R_BUFS = 2
T_BUFS = 2
W_SCALE = 4096.0
DF8 = mybir.dt.float8e4
F8NP = mybir.dt.np(mybir.dt.float8e4)


def _leaf_cols(ops, cheb_cols):
    if not FP8_LEAVES:
        return []
    read = set()
    for op in ops:
        if op[0] == "act":
            read.add(op[2])
        elif op[0] == "stt":
            read.add(op[2])
            read.add(op[5])
        else:
            read.add(op[2])
            read.add(op[3])
    return [c for c in cheb_cols if c not in read]


def _build_nc(S, niter=1, ablate=None):
    # ablate: None (normal), "producers" (memset columns once; PE/DMA path only)
    ops, cheb_cols, _ = _solve_basis(S, range(BASE_DEGREES + 1))
    leaves = _leaf_cols(ops, cheb_cols)
    bf_blocks = [c for c in cheb_cols if c not in leaves] + ["silu"]
    n_kk = 2 * len(bf_blocks)              # bf16 k-tiles of 128
    n8 = len(leaves)                       # fp8 DoubleRow blocks (256 k-rows each)

    nc = bacc.Bacc("TRN2", target_bir_lowering=False, debug=False,
                   num_devices=N_CORES)
    # register const APs for the activation bias values we use (only 0.0/1.0
    # are pre-registered); mirrors Bass.__init__'s register_const_ap.
    bias_consts = sorted({op[5] for op in ops if op[0] == "act"} - {0.0})
    for v in bias_consts:
        t_c = nc.alloc_sbuf_tensor(f"const-f32-{v}", [128, 1], F32)
        nc.gpsimd.memset(t_c.ap(), v)
        nc.const_aps.aps[(F32, v)] = t_c.ap()
    if bias_consts:
        nc.all_engine_barrier()
    x_d = nc.dram_tensor("xt", [128, 2 * B_LOC], F32, kind="ExternalInput").ap()
    w_d = nc.dram_tensor("w", [128, n_kk * O_DIM], DBF16, kind="ExternalInput").ap()
    if n8:
        w8_d = nc.dram_tensor("w8", [128, n8 * 2 * O_DIM], DF8,
                              kind="ExternalInput").ap()
    b_d = nc.dram_tensor("bias", [O_DIM, 1], F32, kind="ExternalInput").ap()
    o_d = nc.dram_tensor("out", [O_DIM, B_LOC], F32, kind="ExternalOutput").ap()

    with tile.TileContext(nc) as tc, ExitStack() as ctx:
        cpool = ctx.enter_context(tc.tile_pool(name="const", bufs=1))
        xpool = ctx.enter_context(tc.tile_pool(name="x", bufs=3))
        rpool = ctx.enter_context(tc.tile_pool(name="r", bufs=R_BUFS))
        tpool = ctx.enter_context(tc.tile_pool(name="tmp", bufs=T_BUFS))
        opool = ctx.enter_context(tc.tile_pool(name="o", bufs=8))
        pspool = ctx.enter_context(tc.tile_pool(name="ps", bufs=8, space="PSUM"))

        wt = cpool.tile([128, n_kk * O_DIM], DBF16)
        nc.sync.dma_start(wt[:], w_d[:])
        if n8:
            w8t = cpool.tile([128, n8 * 2 * O_DIM], DF8)
            nc.sync.dma_start(w8t[:], w8_d[:])
            w8v = w8t[:].rearrange("p (b r o) -> p b r o", r=2, o=O_DIM)
        bt = []
        for m in range(2):
            b_tile = cpool.tile([128, 1], F32, tag=f"bias{m}")
            nc.sync.dma_start(b_tile[:], b_d[m * 128:(m + 1) * 128, :])
            bt.append(b_tile)

        chunks = []
        off = 0
        for bc in CHUNK_SIZES:
            chunks.append((off, bc))
            off += bc
        const_tiles = None
        if ablate == "producers":
            const_tiles = {}
            for nm in (["silu"] + cheb_cols):
                ct = cpool.tile([128, 2 * BC_MAX],
                                DF8 if nm in leaves else DBF16,
                                tag=f"ab_{nm}", name=f"ab_{nm}")
                nc.vector.memset(ct[:], 0.5)
                const_tiles[nm] = ct
        for it in range(niter):
            for ci, (off, bc) in enumerate(chunks):
                cc = f"{it}_{ci}"
                xt = xpool.tile([128, 2 * bc], F32, tag="xt", name=f"xt{cc}")
                nc.sync.dma_start(xt[:], x_d[:, 2 * off: 2 * (off + bc)])

                tiles = {"x": xt}
                if ablate == "producers":
                    tiles.update(const_tiles)
                else:
                    for op in ops:
                        kind, name = op[0], op[1]
                        is_col = (name in bf_blocks) or (name in leaves)
                        pool = rpool if is_col else tpool
                        dt_col = DF8 if name in leaves else DBF16
                        t = pool.tile([128, 2 * bc], dt_col, tag=name,
                                      name=f"{name}_{cc}")
                        if kind == "act":
                            _, _, src, func, scale, bias_v = op
                            nc.scalar.activation(t[:], tiles[src][:], func,
                                                 bias=bias_v, scale=scale)
                        elif kind == "stt":
                            _, _, in0, scalar, op0, in1, op1 = op
                            nc.vector.scalar_tensor_tensor(t[:], tiles[in0][:], scalar,
                                                           tiles[in1][:], op0, op1)
                        else:  # tt
                            _, _, in0, in1, alu = op
                            nc.vector.tensor_tensor(t[:], tiles[in0][:],
                                                    tiles[in1][:], alu)
                        tiles[name] = t

                nsubs = [(s, min(512, bc - s)) for s in range(0, bc, 512)]
                n_mm = n_kk + n8
                for m in range(2):
                    for so, sn in nsubs:
                        ps = pspool.tile([128, sn], F32, tag="ps",
                                         name=f"ps{cc}_{m}_{so}")
                        mi = 0
                        for kk in range(n_kk):
                            j, h = kk // 2, kk % 2
                            rt = tiles[bf_blocks[j]]
                            nc.tensor.matmul(
                                ps[:],
                                wt[:, kk * O_DIM + m * 128: kk * O_DIM + (m + 1) * 128],
                                rt[:, h * bc + so: h * bc + so + sn],
                                start=(mi == 0), stop=(mi == n_mm - 1),
                            )
                            mi += 1
                        for b8, lf in enumerate(leaves):
                            rt = tiles[lf]
                            rhs = rt[:].rearrange("p (r n) -> p r n", r=2)[:, :, so:so + sn]
                            nc.tensor.matmul(
                                ps[:],
                                w8v[:, b8, :, m * 128:(m + 1) * 128],
                                rhs,
                                start=(mi == 0), stop=(mi == n_mm - 1),
                                perf_mode=mybir.MatmulPerfMode.DoubleRow,
                            )
                            mi += 1
                        ot = opool.tile([128, sn], F32, tag="ot",
                                        name=f"ot{cc}_{m}_{so}")
                        nc.vector.tensor_scalar(ot[:], ps[:], 1.0 / W_SCALE,
                                                bt[m][:], ALU.mult, ALU.add)
                        nc.sync.dma_start(
                            o_d[m * 128:(m + 1) * 128, off + so: off + so + sn],
                            ot[:])

    nc.compile()
    return nc


_NC_CACHE = {}


def _get_nc(S, niter=1):
    key = (tuple(S), niter)
    if key not in _NC_CACHE:
        _NC_CACHE[key] = _build_nc(S, niter)
    return _NC_CACHE[key]


# ---------------- host wrapper ----------------

def _prepare(x, logits, cheby_coeffs, base_weight, gating_weights, arange):
    x = np.asarray(x, dtype=np.float32)
    logits = np.asarray(logits, dtype=np.float32)
    cheby_coeffs = np.asarray(cheby_coeffs, dtype=np.float32)
    base_weight = np.asarray(base_weight, dtype=np.float32)
    gating_weights = np.asarray(gating_weights, dtype=np.float32)
    arange = np.asarray(arange)

    # top-k routing (host; 8 numbers). Matches jax.lax.top_k ordering.
    order = np.argsort(-logits, kind="stable")[:TOPK]
    topk_vals = 1.0 / (1.0 + np.exp(-logits[order].astype(np.float64)))
    gate = gating_weights.astype(np.float64).copy()
    sel = order + BASE_DEGREES + 1
    gate[sel] = topk_vals
    S = sorted(int(v) for v in sel)

    low = sorted(int(v) for v in arange)   # normally [0..8]
    ops, cheb_cols, X = _solve_basis(S, low)
    _, vec = _recipe(S)
    leaves = _leaf_cols(ops, cheb_cols)
    bf_cheb = [c for c in cheb_cols if c not in leaves]

    # true (f64) weight blocks per column via the basis solve
    G = {n: gate[n] * cheby_coeffs[:, :, n].astype(np.float64) for n in set(low) | set(S)}
    Wtrue = {"silu": base_weight.T.astype(np.float64)}
    bias = np.zeros(O_DIM, dtype=np.float64)
    for j, cn in enumerate(cheb_cols):
        Wb = np.zeros((I_DIM, O_DIM), dtype=np.float64)
        for n, sol in X.items():
            coef = sol[1 + j]
            if coef != 0.0 and n in G:
                Wb += coef * G[n]
        Wtrue[cn] = Wb
    for n, sol in X.items():
        if sol[0] != 0.0 and n in G:
            bias += sol[0] * G[n].sum(axis=0)

    # fp8-quantize leaf blocks (scaled); error-feedback: fold the recoverable
    # Chebyshev content of the quantization residual into the bf16 blocks.
    W8q = {}
    Wadd = {cn: 0.0 for cn in bf_cheb}
    if leaves:
        Alow = np.zeros((40, 1 + len(bf_cheb)))
        Alow[0, 0] = 1.0
        for j, cn in enumerate(bf_cheb):
            Alow[:, 1 + j] = vec[cn]
        comp = {}
        for cn in leaves:
            q = (Wtrue[cn] * W_SCALE).astype(np.float32).astype(F8NP)
            W8q[cn] = q
            dW = Wtrue[cn] - q.astype(np.float64) / W_SCALE
            v = vec[cn]
            for n in np.nonzero(np.abs(v) > 1e-9)[0]:
                if n == 0:
                    bias += v[0] * dW.sum(axis=0)
                else:
                    comp[int(n)] = comp.get(int(n), 0.0) + v[n] * dW
        for n, V in comp.items():
            sol, *_ = np.linalg.lstsq(Alow, _e(n), rcond=None)
            if np.abs(Alow @ sol - _e(n)).max() > 1e-9:
                continue  # degree only carried by an fp8 column; leave as is
            bias += sol[0] * V.sum(axis=0)
            for j, cn in enumerate(bf_cheb):
                if sol[1 + j] != 0.0:
                    Wadd[cn] = Wadd[cn] + sol[1 + j] * V

    # device layouts (all weights scaled by W_SCALE; evac descales)
    bf_blocks = bf_cheb + ["silu"]
    Wsb = np.empty((128, 2 * len(bf_blocks), O_DIM), dtype=np.float32)
    for j, cn in enumerate(bf_blocks):
        Wf = ((Wtrue[cn] + (Wadd.get(cn, 0.0))) * W_SCALE).astype(np.float32)
        Wsb[:, 2 * j + 0, :] = Wf[0:128, :]
        Wsb[:, 2 * j + 1, :] = Wf[128:256, :]
    w_np = Wsb.reshape(128, 2 * len(bf_blocks) * O_DIM).astype(BF16)
    w8_np = None
    if leaves:
        W8sb = np.empty((128, 2 * len(leaves), O_DIM), dtype=F8NP)
        for b8, cn in enumerate(leaves):
            W8sb[:, 2 * b8 + 0, :] = W8q[cn][0:128, :]
            W8sb[:, 2 * b8 + 1, :] = W8q[cn][128:256, :]
        w8_np = W8sb.reshape(128, 2 * len(leaves) * O_DIM)
    bias_np = bias.astype(np.float32).reshape(O_DIM, 1)
    return S, w_np, w8_np, bias_np


def _make_xt(xl):
    """xt[p, 2*off + h*bc + bb] = xl[off+bb, 128*h+p] for each chunk (off, bc)."""
    out = np.empty((128, 2 * B_LOC), dtype=np.float32)
    off = 0
    for bc in CHUNK_SIZES:
        blk = xl[off:off + bc, :].reshape(bc, 2, 128).transpose(2, 1, 0)
        out[:, 2 * off: 2 * (off + bc)] = blk.reshape(128, 2 * bc)
        off += bc
    return out


def _make_in_maps(x, w_np, w8_np, bias_np):
    in_maps = []
    for c in range(N_CORES):
        m = {"xt": _make_xt(x[c * B_LOC:(c + 1) * B_LOC, :]),
             "w": w_np, "bias": bias_np}
        if w8_np is not None:
            m["w8"] = w8_np
        in_maps.append(m)
    return in_maps


def kernel(x, t, logits, cheby_coeffs, base_weight, gating_weights, arange):
    x = np.asarray(x, dtype=np.float32)
    S, w_np, w8_np, bias_np = _prepare(x, logits, cheby_coeffs, base_weight,
                                       gating_weights, arange)
    nc = _get_nc(S)
    in_maps = _make_in_maps(x, w_np, w8_np, bias_np)
    res = run_bass_kernel_spmd(nc, in_maps, core_ids=list(range(N_CORES)))
    y = np.empty((BATCH, O_DIM), dtype=np.float32)
    for c in range(N_CORES):
        y[c * B_LOC:(c + 1) * B_LOC, :] = res.results[c]["out"].T
    return y


# revision 32
# speedup vs baseline: 1.2640x; 1.1592x over previous
"""AChebyKANLinear forward on 8 TRN2 NeuronCores (data-parallel over batch).

y = silu(x) @ W_base^T + einsum('bid,iod->bo', cos(n_d * arccos(tanh x)), gated_coeffs)

Key identities used:
  cos(n*arccos(c)) = T_n(c)  (Chebyshev), c = tanh(x)
  -> no trig needed on device. Device computes 13 "columns" per feature:
     silu(x), and 12 cheap polynomials of c whose exact Chebyshev-basis
     expansion is tracked symbolically on the host; the host solves a small
     linear system to fold the change of basis into the matmul weights.
  Even T_2m come from ACT Square ops (T_2m+1 = 2*T_m^2), odd ones from single
  fused DVE scalar_tensor_tensor ops. All columns bf16; one big
  [4096, 3328] @ [3328, 256] GEMM per core on TensorE (fp32 PSUM accum).

Top-k routing over the 8 logits is computed on the host (it is 8 numbers);
the 4 selected high degrees are baked into the compiled graph.
"""

import numpy as np
import ml_dtypes
from contextlib import ExitStack

import concourse.bass as bass
import concourse.tile as tile
from concourse import bacc, mybir
from concourse.bass_utils import run_bass_kernel_spmd

BF16 = ml_dtypes.bfloat16

N_CORES = 8
BATCH, I_DIM, O_DIM = 32768, 256, 256
B_LOC = BATCH // N_CORES          # 4096
# graduated batch chunks: small first chunks shorten the pipeline fill before
# TensorE has all 13 columns of chunk 0; steady state runs at 1024.
CHUNK_SIZES = [256, 256, 512, 1024, 1024, 1024]
assert sum(CHUNK_SIZES) == B_LOC
BC_MAX = max(CHUNK_SIZES)
DEGREE = 16
BASE_DEGREES = 8
TOPK = 4

SQ2 = float(np.sqrt(2.0))

A = mybir.ActivationFunctionType
ALU = mybir.AluOpType
F32 = mybir.dt.float32
DBF16 = mybir.dt.bfloat16


# ---------------- symbolic Chebyshev algebra (host, exact) ----------------

def _chmul(a, b):
    out = np.zeros(40)
    nz_a = np.nonzero(a)[0]
    nz_b = np.nonzero(b)[0]
    for i in nz_a:
        for j in nz_b:
            p = a[i] * b[j] * 0.5
            out[i + j] += p
            out[abs(i - j)] += p
    return out


def _e(n):
    v = np.zeros(40)
    v[n] = 1.0
    return v


def _recipe(S):
    """Build the per-chunk op recipe and each column's Chebyshev expansion.

    Returns (ops, colvec) where ops is a list of
      ('act', name, src, func, scale, bias) or
      ('stt', name, in0, scalar, op0, in1, op1) or
      ('tt',  name, in0, in1, op)
    and colvec maps tile name -> length-40 Chebyshev coefficient vector.
    """
    ops = []
    vec = {}

    def act(name, src, func, scale=1.0, bias=0.0):
        ops.append(("act", name, src, func, float(scale), float(bias)))
        if func == A.Square:
            aff = vec[src] * scale
            aff[0] += bias
            vec[name] = _chmul(aff, aff)
        elif func == A.Tanh:
            vec[name] = _e(1)
        else:  # Silu: not a Chebyshev column
            vec[name] = None

    def stt(name, in0, scalar, op0, in1, op1):
        ops.append(("stt", name, in0, float(scalar), op0, in1, op1))
        a = vec[in0].copy()
        if op0 == ALU.add:
            a[0] += scalar
        elif op0 == ALU.mult:
            a = a * scalar
        else:
            raise ValueError(op0)
        b = vec[in1]
        if op1 == ALU.mult:
            vec[name] = _chmul(a, b)
        elif op1 == ALU.subtract:
            vec[name] = a - b
        elif op1 == ALU.add:
            vec[name] = a + b
        else:
            raise ValueError(op1)

    def tt(name, in0, in1, op):
        ops.append(("tt", name, in0, in1, op))
        if op == ALU.subtract:
            vec[name] = vec[in0] - vec[in1]
        elif op == ALU.add:
            vec[name] = vec[in0] + vec[in1]
        elif op == ALU.mult:
            vec[name] = _chmul(vec[in0], vec[in1])
        else:
            raise ValueError(op)

    # c1 first: it unblocks the whole Square chain and every DVE op; silu is
    # only consumed by the last K-blocks, so it is produced late.
    act("c1", "x", A.Tanh)
    act("c2", "c1", A.Square, SQ2)            # T2 + 1
    stt("c3", "c2", -1.5, ALU.add, "c1", ALU.mult)   # (c2-1.5)*c1 = T3/2
    act("c4", "c2", A.Square, SQ2, -SQ2)      # T4 + 1
    stt("c5", "c4", -1.0, ALU.add, "c1", ALU.mult)   # T4*T1 = (T5+T3)/2
    stt("c6", "c3", 2.0, ALU.mult, "c3", ALU.mult)   # 2*c3^2 = (T6+1)/2  (DVE)
    stt("c7", "c4", -1.0, ALU.add, "c3", ALU.mult)   # T4*T3/2 = (T7+T1)/4
    act("c8", "c4", A.Square, SQ2, -SQ2)      # T8 + 1
    act("silu", "x", A.Silu)
    for n in S:
        if n == 9:
            stt("c9", "c8", -1.0, ALU.add, "c1", ALU.mult)    # T8*T1
        elif n == 10:
            act("c10", "c5", A.Square, SQ2)                    # 2*c5^2
        elif n == 11:
            stt("c11", "c8", -1.0, ALU.add, "c3", ALU.mult)   # T8*T3/2
        elif n == 12:
            act("c12", "c6", A.Square, 2.0 * SQ2, -SQ2)        # 2*(2*c6-1)^2 = T12+1
        elif n == 13:
            tt("d53", "c5", "c3", ALU.subtract)                # T5/2
            stt("c13", "c8", -1.0, ALU.add, "d53", ALU.mult)  # T8*T5/2
        elif n == 14:
            stt("c14", "c8", -1.0, ALU.add, "c6", ALU.mult)   # T8*(T6+1)/2
        elif n == 15:
            stt("t7p", "c7", 4.0, ALU.mult, "c1", ALU.subtract)  # T7
            stt("c15", "c8", -1.0, ALU.add, "t7p", ALU.mult)     # T8*T7
        elif n == 16:
            act("c16", "c8", A.Square, SQ2, -SQ2)              # T16+1
        else:
            raise ValueError(n)
    return ops, vec


def _solve_basis(S, low_degrees):
    """Solve for X s.t. sum_col X[col,n]*colvec[col] == e_n for each needed n.

    Columns: 'bias' (the constant 1) + the 12 device Chebyshev columns.
    Returns (ops, matmul_cols, X) with X keyed [col][n].
    """
    ops, vec = _recipe(S)
    cheb_cols = ["c1", "c2", "c3", "c4", "c5", "c6", "c7", "c8"] + [f"c{n}" for n in S]
    needed = sorted(set(int(n) for n in low_degrees) | set(S))
    Amat = np.zeros((40, 1 + len(cheb_cols)))
    Amat[0, 0] = 1.0  # bias column = T_0
    for j, cn in enumerate(cheb_cols):
        Amat[:, 1 + j] = vec[cn]
    X = {}
    for n in needed:
        sol, res, rank, _ = np.linalg.lstsq(Amat, _e(n), rcond=None)
        err = np.abs(Amat @ sol - _e(n)).max()
        assert err < 1e-9, f"basis solve failed for degree {n}: {err}"
        X[n] = sol  # [1+len(cheb_cols)]
    return ops, cheb_cols, X


# ---------------- device graph ----------------

# fp8 "leaf" columns: columns no chain op reads can be written float8_e4m3
# directly and contracted with DoubleRow matmuls (2 k-rows per instruction).
# All weights are scaled by W_SCALE on host (so fp8 weights avoid subnormals);
# the PSUM is descaled during the bias-add evacuation.
# Measured on HW: DoubleRow groups are 1.4x faster in an isolated microbench,
# but in this kernel the 256-col non-FWL LDWEIGHTS exposure cancels the gain
# (A/B medians 94us fp8 vs 91us bf16) while costing rel-err 8e-3 vs 4.2e-3.
# Kept implemented but disabled.
FP8_LEAVES = False
R_BUFS = 2
T_BUFS = 2
W_SCALE = 4096.0
DF8 = mybir.dt.float8e4
F8NP = mybir.dt.np(mybir.dt.float8e4)


def _leaf_cols(ops, cheb_cols):
    if not FP8_LEAVES:
        return []
    read = set()
    for op in ops:
        if op[0] == "act":
            read.add(op[2])
        elif op[0] == "stt":
            read.add(op[2])
            read.add(op[5])
        else:
            read.add(op[2])
            read.add(op[3])
    return [c for c in cheb_cols if c not in read]


def _build_nc(S, niter=1, ablate=None):
    # ablate: None (normal), "producers" (memset columns once; PE/DMA path only)
    ops, cheb_cols, _ = _solve_basis(S, range(BASE_DEGREES + 1))
    leaves = _leaf_cols(ops, cheb_cols)
    bf_blocks = [c for c in cheb_cols if c not in leaves] + ["silu"]
    n_kk = 2 * len(bf_blocks)              # bf16 k-tiles of 128
    n8 = len(leaves)                       # fp8 DoubleRow blocks (256 k-rows each)

    nc = bacc.Bacc("TRN2", target_bir_lowering=False, debug=False,
                   num_devices=N_CORES)
    # register const APs for the activation bias values we use (only 0.0/1.0
    # are pre-registered); mirrors Bass.__init__'s register_const_ap.
    bias_consts = sorted({op[5] for op in ops if op[0] == "act"} - {0.0})
    for v in bias_consts:
        t_c = nc.alloc_sbuf_tensor(f"const-f32-{v}", [128, 1], F32)
        nc.gpsimd.memset(t_c.ap(), v)
        nc.const_aps.aps[(F32, v)] = t_c.ap()
    if bias_consts:
        nc.all_engine_barrier()
    x_d = nc.dram_tensor("xt", [128, 2 * B_LOC], F32, kind="ExternalInput").ap()
    w_d = nc.dram_tensor("w", [128, n_kk * O_DIM], DBF16, kind="ExternalInput").ap()
    if n8:
        w8_d = nc.dram_tensor("w8", [128, n8 * 2 * O_DIM], DF8,
                              kind="ExternalInput").ap()
    b_d = nc.dram_tensor("bias", [O_DIM, 1], F32, kind="ExternalInput").ap()
    o_d = nc.dram_tensor("out", [O_DIM, B_LOC], F32, kind="ExternalOutput").ap()

    with tile.TileContext(nc) as tc, ExitStack() as ctx:
        cpool = ctx.enter_context(tc.tile_pool(name="const", bufs=1))
        xpool = ctx.enter_context(tc.tile_pool(name="x", bufs=3))
        rpool = ctx.enter_context(tc.tile_pool(name="r", bufs=R_BUFS))
        tpool = ctx.enter_context(tc.tile_pool(name="tmp", bufs=T_BUFS))
        opool = ctx.enter_context(tc.tile_pool(name="o", bufs=8))
        pspool = ctx.enter_context(tc.tile_pool(name="ps", bufs=8, space="PSUM"))

        wt = cpool.tile([128, n_kk * O_DIM], DBF16)
        nc.sync.dma_start(wt[:], w_d[:])
        if n8:
            w8t = cpool.tile([128, n8 * 2 * O_DIM], DF8)
            nc.sync.dma_start(w8t[:], w8_d[:])
            w8v = w8t[:].rearrange("p (b r o) -> p b r o", r=2, o=O_DIM)
        bt = []
        for m in range(2):
            b_tile = cpool.tile([128, 1], F32, tag=f"bias{m}")
            nc.sync.dma_start(b_tile[:], b_d[m * 128:(m + 1) * 128, :])
            bt.append(b_tile)

        chunks = []
        off = 0
        for bc in CHUNK_SIZES:
            chunks.append((off, bc))
            off += bc
        const_tiles = None
        if ablate == "producers":
            const_tiles = {}
            for nm in (["silu"] + cheb_cols):
                ct = cpool.tile([128, 2 * BC_MAX],
                                DF8 if nm in leaves else DBF16,
                                tag=f"ab_{nm}", name=f"ab_{nm}")
                nc.vector.memset(ct[:], 0.5)
                const_tiles[nm] = ct
        for it in range(niter):
            for ci, (off, bc) in enumerate(chunks):
                cc = f"{it}_{ci}"
                xt = xpool.tile([128, 2 * bc], F32, tag="xt", name=f"xt{cc}")
                nc.sync.dma_start(xt[:], x_d[:, 2 * off: 2 * (off + bc)])

                tiles = {"x": xt}
                if ablate == "producers":
                    tiles.update(const_tiles)
                else:
                    for op in ops:
                        kind, name = op[0], op[1]
                        is_col = (name in bf_blocks) or (name in leaves)
                        pool = rpool if is_col else tpool
                        dt_col = DF8 if name in leaves else DBF16
                        t = pool.tile([128, 2 * bc], dt_col, tag=name,
                                      name=f"{name}_{cc}")
                        if kind == "act":
                            _, _, src, func, scale, bias_v = op
                            nc.scalar.activation(t[:], tiles[src][:], func,
                                                 bias=bias_v, scale=scale)
                        elif kind == "stt":
                            _, _, in0, scalar, op0, in1, op1 = op
                            nc.vector.scalar_tensor_tensor(t[:], tiles[in0][:], scalar,
                                                           tiles[in1][:], op0, op1)
                        else:  # tt
                            _, _, in0, in1, alu = op
                            nc.vector.tensor_tensor(t[:], tiles[in0][:],
                                                    tiles[in1][:], alu)
                        tiles[name] = t

                nsubs = [(s, min(512, bc - s)) for s in range(0, bc, 512)]
                n_mm = n_kk + n8
                for m in range(2):
                    for so, sn in nsubs:
                        ps = pspool.tile([128, sn], F32, tag="ps",
                                         name=f"ps{cc}_{m}_{so}")
                        mi = 0
                        for kk in range(n_kk):
                            j, h = kk // 2, kk % 2
                            rt = tiles[bf_blocks[j]]
                            nc.tensor.matmul(
                                ps[:],
                                wt[:, kk * O_DIM + m * 128: kk * O_DIM + (m + 1) * 128],
                                rt[:, h * bc + so: h * bc + so + sn],
                                start=(mi == 0), stop=(mi == n_mm - 1),
                            )
                            mi += 1
                        for b8, lf in enumerate(leaves):
                            rt = tiles[lf]
                            rhs = rt[:].rearrange("p (r n) -> p r n", r=2)[:, :, so:so + sn]
                            nc.tensor.matmul(
                                ps[:],
                                w8v[:, b8, :, m * 128:(m + 1) * 128],
                                rhs,
                                start=(mi == 0), stop=(mi == n_mm - 1),
                                perf_mode=mybir.MatmulPerfMode.DoubleRow,
                            )
                            mi += 1
                        ot = opool.tile([128, sn], F32, tag="ot",
                                        name=f"ot{cc}_{m}_{so}")
                        nc.vector.tensor_scalar(ot[:], ps[:], 1.0 / W_SCALE,
                                                bt[m][:], ALU.mult, ALU.add)
                        nc.sync.dma_start(
                            o_d[m * 128:(m + 1) * 128, off + so: off + so + sn],
                            ot[:])

    nc.compile()
    return nc


_NC_CACHE = {}


def _get_nc(S, niter=1):
    key = (tuple(S), niter)
    if key not in _NC_CACHE:
        _NC_CACHE[key] = _build_nc(S, niter)
    return _NC_CACHE[key]


# ---------------- host wrapper ----------------

def _prepare(x, logits, cheby_coeffs, base_weight, gating_weights, arange):
    x = np.asarray(x, dtype=np.float32)
    logits = np.asarray(logits, dtype=np.float32)
    cheby_coeffs = np.asarray(cheby_coeffs, dtype=np.float32)
    base_weight = np.asarray(base_weight, dtype=np.float32)
    gating_weights = np.asarray(gating_weights, dtype=np.float32)
    arange = np.asarray(arange)

    # top-k routing (host; 8 numbers). Matches jax.lax.top_k ordering.
    order = np.argsort(-logits, kind="stable")[:TOPK]
    topk_vals = 1.0 / (1.0 + np.exp(-logits[order].astype(np.float64)))
    gate = gating_weights.astype(np.float64).copy()
    sel = order + BASE_DEGREES + 1
    gate[sel] = topk_vals
    S = sorted(int(v) for v in sel)

    low = sorted(int(v) for v in arange)   # normally [0..8]
    ops, cheb_cols, X = _solve_basis(S, low)
    _, vec = _recipe(S)
    leaves = _leaf_cols(ops, cheb_cols)
    bf_cheb = [c for c in cheb_cols if c not in leaves]

    # true (f64) weight blocks per column via the basis solve
    G = {n: gate[n] * cheby_coeffs[:, :, n].astype(np.float64) for n in set(low) | set(S)}
    Wtrue = {"silu": base_weight.T.astype(np.float64)}
    bias = np.zeros(O_DIM, dtype=np.float64)
    for j, cn in enumerate(cheb_cols):
        Wb = np.zeros((I_DIM, O_DIM), dtype=np.float64)
        for n, sol in X.items():
            coef = sol[1 + j]
            if coef != 0.0 and n in G:
                Wb += coef * G[n]
        Wtrue[cn] = Wb
    for n, sol in X.items():
        if sol[0] != 0.0 and n in G:
            bias += sol[0] * G[n].sum(axis=0)

    # fp8-quantize leaf blocks (scaled); error-feedback: fold the recoverable
    # Chebyshev content of the quantization residual into the bf16 blocks.
    W8q = {}
    Wadd = {cn: 0.0 for cn in bf_cheb}
    if leaves:
        Alow = np.zeros((40, 1 + len(bf_cheb)))
        Alow[0, 0] = 1.0
        for j, cn in enumerate(bf_cheb):
            Alow[:, 1 + j] = vec[cn]
        comp = {}
        for cn in leaves:
            q = (Wtrue[cn] * W_SCALE).astype(np.float32).astype(F8NP)
            W8q[cn] = q
            dW = Wtrue[cn] - q.astype(np.float64) / W_SCALE
            v = vec[cn]
            for n in np.nonzero(np.abs(v) > 1e-9)[0]:
                if n == 0:
                    bias += v[0] * dW.sum(axis=0)
                else:
                    comp[int(n)] = comp.get(int(n), 0.0) + v[n] * dW
        for n, V in comp.items():
            sol, *_ = np.linalg.lstsq(Alow, _e(n), rcond=None)
            if np.abs(Alow @ sol - _e(n)).max() > 1e-9:
                continue  # degree only carried by an fp8 column; leave as is
            bias += sol[0] * V.sum(axis=0)
            for j, cn in enumerate(bf_cheb):
                if sol[1 + j] != 0.0:
                    Wadd[cn] = Wadd[cn] + sol[1 + j] * V

    # device layouts (all weights scaled by W_SCALE; evac descales)
    bf_blocks = bf_cheb + ["silu"]
    Wsb = np.empty((128, 2 * len(bf_blocks), O_DIM), dtype=np.float32)
    for j, cn in enumerate(bf_blocks):
        Wf = ((Wtrue[cn] + (Wadd.get(cn, 0.0))) * W_SCALE).astype(np.float32)
        Wsb[:, 2 * j + 0, :] = Wf[0:128, :]
        Wsb[:, 2 * j + 1, :] = Wf[128:256, :]
    w_np = Wsb.reshape(128, 2 * len(bf_blocks) * O_DIM).astype(BF16)
    w8_np = None
    if leaves:
        W8sb = np.empty((128, 2 * len(leaves), O_DIM), dtype=F8NP)
        for b8, cn in enumerate(leaves):
            W8sb[:, 2 * b8 + 0, :] = W8q[cn][0:128, :]
            W8sb[:, 2 * b8 + 1, :] = W8q[cn][128:256, :]
        w8_np = W8sb.reshape(128, 2 * len(leaves) * O_DIM)
    bias_np = bias.astype(np.float32).reshape(O_DIM, 1)
    return S, w_np, w8_np, bias_np


def _make_xt(xl):
    """xt[p, 2*off + h*bc + bb] = xl[off+bb, 128*h+p] for each chunk (off, bc)."""
    out = np.empty((128, 2 * B_LOC), dtype=np.float32)
    off = 0
    for bc in CHUNK_SIZES:
        blk = xl[off:off + bc, :].reshape(bc, 2, 128).transpose(2, 1, 0)
        out[:, 2 * off: 2 * (off + bc)] = blk.reshape(128, 2 * bc)
        off += bc
    return out


def _make_in_maps(x, w_np, w8_np, bias_np):
    in_maps = []
    for c in range(N_CORES):
        m = {"xt": _make_xt(x[c * B_LOC:(c + 1) * B_LOC, :]),
             "w": w_np, "bias": bias_np}
        if w8_np is not None:
            m["w8"] = w8_np
        in_maps.append(m)
    return in_maps


def kernel(x, t, logits, cheby_coeffs, base_weight, gating_weights, arange):
    x = np.asarray(x, dtype=np.float32)
    S, w_np, w8_np, bias_np = _prepare(x, logits, cheby_coeffs, base_weight,
                                       gating_weights, arange)
    nc = _get_nc(S)
    in_maps = _make_in_maps(x, w_np, w8_np, bias_np)
    res = run_bass_kernel_spmd(nc, in_maps, core_ids=list(range(N_CORES)))
    y = np.empty((BATCH, O_DIM), dtype=np.float32)
    for c in range(N_CORES):
        y[c * B_LOC:(c + 1) * B_LOC, :] = res.results[c]["out"].T
    return y


# revision 35
# speedup vs baseline: 1.3658x; 1.0805x over previous
"""AChebyKANLinear forward on 8 TRN2 NeuronCores (data-parallel over batch).

y = silu(x) @ W_base^T + einsum('bid,iod->bo', cos(n_d * arccos(tanh x)), gated_coeffs)

Key identities used:
  cos(n*arccos(c)) = T_n(c)  (Chebyshev), c = tanh(x)
  -> no trig needed on device. Device computes 13 "columns" per feature:
     silu(x), and 12 cheap polynomials of c whose exact Chebyshev-basis
     expansion is tracked symbolically on the host; the host solves a small
     linear system to fold the change of basis into the matmul weights.
  Even T_2m come from ACT Square ops (T_2m+1 = 2*T_m^2), odd ones from single
  fused DVE scalar_tensor_tensor ops. All columns bf16; one big
  [4096, 3328] @ [3328, 256] GEMM per core on TensorE (fp32 PSUM accum).

Top-k routing over the 8 logits is computed on the host (it is 8 numbers);
the 4 selected high degrees are baked into the compiled graph.
"""

import numpy as np
import ml_dtypes
from contextlib import ExitStack

import concourse.bass as bass
import concourse.tile as tile
from concourse import bacc, mybir
from concourse.bass_utils import run_bass_kernel_spmd

BF16 = ml_dtypes.bfloat16

N_CORES = 8
BATCH, I_DIM, O_DIM = 32768, 256, 256
B_LOC = BATCH // N_CORES          # 4096
# graduated batch chunks: small first chunks shorten the pipeline fill before
# TensorE has all 13 columns of chunk 0; steady state runs at 1024.
CHUNK_SIZES = [256, 256, 512, 1024, 1024, 1024]
assert sum(CHUNK_SIZES) == B_LOC
BC_MAX = max(CHUNK_SIZES)
DEGREE = 16
BASE_DEGREES = 8
TOPK = 4

SQ2 = float(np.sqrt(2.0))

A = mybir.ActivationFunctionType
ALU = mybir.AluOpType
F32 = mybir.dt.float32
DBF16 = mybir.dt.bfloat16


# ---------------- symbolic Chebyshev algebra (host, exact) ----------------

def _chmul(a, b):
    out = np.zeros(40)
    nz_a = np.nonzero(a)[0]
    nz_b = np.nonzero(b)[0]
    for i in nz_a:
        for j in nz_b:
            p = a[i] * b[j] * 0.5
            out[i + j] += p
            out[abs(i - j)] += p
    return out


def _e(n):
    v = np.zeros(40)
    v[n] = 1.0
    return v


def _recipe(S):
    """Build the per-chunk op recipe and each column's Chebyshev expansion.

    Returns (ops, colvec) where ops is a list of
      ('act', name, src, func, scale, bias) or
      ('stt', name, in0, scalar, op0, in1, op1) or
      ('tt',  name, in0, in1, op)
    and colvec maps tile name -> length-40 Chebyshev coefficient vector.
    """
    ops = []
    vec = {}

    def act(name, src, func, scale=1.0, bias=0.0):
        ops.append(("act", name, src, func, float(scale), float(bias)))
        if func == A.Square:
            aff = vec[src] * scale
            aff[0] += bias
            vec[name] = _chmul(aff, aff)
        elif func == A.Tanh:
            vec[name] = _e(1)
        else:  # Silu: not a Chebyshev column
            vec[name] = None

    def stt(name, in0, scalar, op0, in1, op1):
        ops.append(("stt", name, in0, float(scalar), op0, in1, op1))
        a = vec[in0].copy()
        if op0 == ALU.add:
            a[0] += scalar
        elif op0 == ALU.mult:
            a = a * scalar
        else:
            raise ValueError(op0)
        b = vec[in1]
        if op1 == ALU.mult:
            vec[name] = _chmul(a, b)
        elif op1 == ALU.subtract:
            vec[name] = a - b
        elif op1 == ALU.add:
            vec[name] = a + b
        else:
            raise ValueError(op1)

    def tt(name, in0, in1, op):
        ops.append(("tt", name, in0, in1, op))
        if op == ALU.subtract:
            vec[name] = vec[in0] - vec[in1]
        elif op == ALU.add:
            vec[name] = vec[in0] + vec[in1]
        elif op == ALU.mult:
            vec[name] = _chmul(vec[in0], vec[in1])
        else:
            raise ValueError(op)

    # c1 first: it unblocks the whole Square chain and every DVE op; silu is
    # only consumed by the last K-blocks, so it is produced late.
    act("c1", "x", A.Tanh)
    act("c2", "c1", A.Square, SQ2)            # T2 + 1
    stt("c3", "c2", -1.5, ALU.add, "c1", ALU.mult)   # (c2-1.5)*c1 = T3/2
    act("c4", "c2", A.Square, SQ2, -SQ2)      # T4 + 1
    stt("c5", "c4", -1.0, ALU.add, "c1", ALU.mult)   # T4*T1 = (T5+T3)/2
    stt("c6", "c3", 2.0, ALU.mult, "c3", ALU.mult)   # 2*c3^2 = (T6+1)/2  (DVE)
    stt("c7", "c4", -1.0, ALU.add, "c3", ALU.mult)   # T4*T3/2 = (T7+T1)/4
    act("c8", "c4", A.Square, SQ2, -SQ2)      # T8 + 1
    act("silu", "x", A.Silu)
    for n in S:
        if n == 9:
            stt("c9", "c8", -1.0, ALU.add, "c1", ALU.mult)    # T8*T1
        elif n == 10:
            act("c10", "c5", A.Square, SQ2)                    # 2*c5^2
        elif n == 11:
            stt("c11", "c8", -1.0, ALU.add, "c3", ALU.mult)   # T8*T3/2
        elif n == 12:
            act("c12", "c6", A.Square, 2.0 * SQ2, -SQ2)        # 2*(2*c6-1)^2 = T12+1
        elif n == 13:
            tt("d53", "c5", "c3", ALU.subtract)                # T5/2
            stt("c13", "c8", -1.0, ALU.add, "d53", ALU.mult)  # T8*T5/2
        elif n == 14:
            stt("c14", "c8", -1.0, ALU.add, "c6", ALU.mult)   # T8*(T6+1)/2
        elif n == 15:
            stt("t7p", "c7", 4.0, ALU.mult, "c1", ALU.subtract)  # T7
            stt("c15", "c8", -1.0, ALU.add, "t7p", ALU.mult)     # T8*T7
        elif n == 16:
            act("c16", "c8", A.Square, SQ2, -SQ2)              # T16+1
        else:
            raise ValueError(n)
    return ops, vec


def _solve_basis(S, low_degrees):
    """Solve for X s.t. sum_col X[col,n]*colvec[col] == e_n for each needed n.

    Columns: 'bias' (the constant 1) + the 12 device Chebyshev columns.
    Returns (ops, matmul_cols, X) with X keyed [col][n].
    """
    ops, vec = _recipe(S)
    cheb_cols = ["c1", "c2", "c3", "c4", "c5", "c6", "c7", "c8"] + [f"c{n}" for n in S]
    needed = sorted(set(int(n) for n in low_degrees) | set(S))
    Amat = np.zeros((40, 1 + len(cheb_cols)))
    Amat[0, 0] = 1.0  # bias column = T_0
    for j, cn in enumerate(cheb_cols):
        Amat[:, 1 + j] = vec[cn]
    X = {}
    for n in needed:
        sol, res, rank, _ = np.linalg.lstsq(Amat, _e(n), rcond=None)
        err = np.abs(Amat @ sol - _e(n)).max()
        assert err < 1e-9, f"basis solve failed for degree {n}: {err}"
        X[n] = sol  # [1+len(cheb_cols)]
    return ops, cheb_cols, X


# ---------------- device graph ----------------

# fp8 "leaf" columns: columns no chain op reads can be written float8_e4m3
# directly and contracted with DoubleRow matmuls (2 k-rows per instruction).
# All weights are scaled by W_SCALE on host (so fp8 weights avoid subnormals);
# the PSUM is descaled during the bias-add evacuation.
# Measured on HW: DoubleRow groups are 1.4x faster in an isolated microbench,
# but in this kernel the 256-col non-FWL LDWEIGHTS exposure cancels the gain
# (A/B medians 94us fp8 vs 91us bf16) while costing rel-err 8e-3 vs 4.2e-3.
# Kept implemented but disabled.
FP8_LEAVES = False
R_BUFS = 2
T_BUFS = 2
W_SCALE = 4096.0
DF8 = mybir.dt.float8e4
F8NP = mybir.dt.np(mybir.dt.float8e4)


def _leaf_cols(ops, cheb_cols):
    if not FP8_LEAVES:
        return []
    read = set()
    for op in ops:
        if op[0] == "act":
            read.add(op[2])
        elif op[0] == "stt":
            read.add(op[2])
            read.add(op[5])
        else:
            read.add(op[2])
            read.add(op[3])
    return [c for c in cheb_cols if c not in read]


def _build_nc(S, niter=1, ablate=None):
    # ablate: None (normal), "producers" (memset columns once; PE/DMA path only)
    ops, cheb_cols, _ = _solve_basis(S, range(BASE_DEGREES + 1))
    leaves = _leaf_cols(ops, cheb_cols)
    bf_blocks = [c for c in cheb_cols if c not in leaves] + ["silu"]
    n_kk = 2 * len(bf_blocks)              # bf16 k-tiles of 128
    n8 = len(leaves)                       # fp8 DoubleRow blocks (256 k-rows each)

    nc = bacc.Bacc("TRN2", target_bir_lowering=False, debug=False,
                   num_devices=N_CORES)
    # register const APs for the activation bias values we use (only 0.0/1.0
    # are pre-registered); mirrors Bass.__init__'s register_const_ap.
    bias_consts = sorted({op[5] for op in ops if op[0] == "act"} - {0.0})
    for v in bias_consts:
        t_c = nc.alloc_sbuf_tensor(f"const-f32-{v}", [128, 1], F32)
        nc.gpsimd.memset(t_c.ap(), v)
        nc.const_aps.aps[(F32, v)] = t_c.ap()
    if bias_consts:
        nc.all_engine_barrier()
    x_d = nc.dram_tensor("xt", [128, 2 * B_LOC], F32, kind="ExternalInput").ap()
    w_d = nc.dram_tensor("w", [128, n_kk * O_DIM], DBF16, kind="ExternalInput").ap()
    if n8:
        w8_d = nc.dram_tensor("w8", [128, n8 * 2 * O_DIM], DF8,
                              kind="ExternalInput").ap()
    b_d = nc.dram_tensor("bias", [O_DIM, 1], F32, kind="ExternalInput").ap()
    o_d = nc.dram_tensor("out", [O_DIM, B_LOC], F32, kind="ExternalOutput").ap()

    with tile.TileContext(nc) as tc, ExitStack() as ctx:
        cpool = ctx.enter_context(tc.tile_pool(name="const", bufs=1))
        xpool = ctx.enter_context(tc.tile_pool(name="x", bufs=3))
        rpool = ctx.enter_context(tc.tile_pool(name="r", bufs=R_BUFS))
        tpool = ctx.enter_context(tc.tile_pool(name="tmp", bufs=T_BUFS))
        opool = ctx.enter_context(tc.tile_pool(name="o", bufs=8))
        pspool = ctx.enter_context(tc.tile_pool(name="ps", bufs=8, space="PSUM"))

        wt = cpool.tile([128, n_kk * O_DIM], DBF16)
        nc.sync.dma_start(wt[:], w_d[:])
        if n8:
            w8t = cpool.tile([128, n8 * 2 * O_DIM], DF8)
            nc.sync.dma_start(w8t[:], w8_d[:])
            w8v = w8t[:].rearrange("p (b r o) -> p b r o", r=2, o=O_DIM)
        bt = []
        for m in range(2):
            b_tile = cpool.tile([128, 1], F32, tag=f"bias{m}")
            nc.sync.dma_start(b_tile[:], b_d[m * 128:(m + 1) * 128, :])
            bt.append(b_tile)

        chunks = []
        off = 0
        for bc in CHUNK_SIZES:
            chunks.append((off, bc))
            off += bc
        const_tiles = None
        if ablate == "producers":
            const_tiles = {}
            for nm in (["silu"] + cheb_cols):
                ct = cpool.tile([128, 2 * BC_MAX],
                                DF8 if nm in leaves else DBF16,
                                tag=f"ab_{nm}", name=f"ab_{nm}")
                nc.vector.memset(ct[:], 0.5)
                const_tiles[nm] = ct
        for it in range(niter):
            for ci, (off, bc) in enumerate(chunks):
                cc = f"{it}_{ci}"
                xt = xpool.tile([128, 2 * bc], F32, tag="xt", name=f"xt{cc}")
                nc.sync.dma_start(xt[:], x_d[:, 2 * off: 2 * (off + bc)])

                tiles = {"x": xt}
                if ablate == "producers":
                    tiles.update(const_tiles)
                else:
                    for op in ops:
                        kind, name = op[0], op[1]
                        is_col = (name in bf_blocks) or (name in leaves)
                        pool = rpool if is_col else tpool
                        dt_col = DF8 if name in leaves else DBF16
                        t = pool.tile([128, 2 * bc], dt_col, tag=name,
                                      name=f"{name}_{cc}")
                        if kind == "act":
                            _, _, src, func, scale, bias_v = op
                            nc.scalar.activation(t[:], tiles[src][:], func,
                                                 bias=bias_v, scale=scale)
                        elif kind == "stt":
                            _, _, in0, scalar, op0, in1, op1 = op
                            nc.vector.scalar_tensor_tensor(t[:], tiles[in0][:], scalar,
                                                           tiles[in1][:], op0, op1)
                        else:  # tt
                            _, _, in0, in1, alu = op
                            nc.vector.tensor_tensor(t[:], tiles[in0][:],
                                                    tiles[in1][:], alu)
                        tiles[name] = t

                nsubs = [(s, min(512, bc - s)) for s in range(0, bc, 512)]
                n_mm = n_kk + n8
                for m in range(2):
                    for so, sn in nsubs:
                        ps = pspool.tile([128, sn], F32, tag="ps",
                                         name=f"ps{cc}_{m}_{so}")
                        mi = 0
                        for kk in range(n_kk):
                            j, h = kk // 2, kk % 2
                            rt = tiles[bf_blocks[j]]
                            nc.tensor.matmul(
                                ps[:],
                                wt[:, kk * O_DIM + m * 128: kk * O_DIM + (m + 1) * 128],
                                rt[:, h * bc + so: h * bc + so + sn],
                                start=(mi == 0), stop=(mi == n_mm - 1),
                            )
                            mi += 1
                        for b8, lf in enumerate(leaves):
                            rt = tiles[lf]
                            rhs = rt[:].rearrange("p (r n) -> p r n", r=2)[:, :, so:so + sn]
                            nc.tensor.matmul(
                                ps[:],
                                w8v[:, b8, :, m * 128:(m + 1) * 128],
                                rhs,
                                start=(mi == 0), stop=(mi == n_mm - 1),
                                perf_mode=mybir.MatmulPerfMode.DoubleRow,
                            )
                            mi += 1
                        ot = opool.tile([128, sn], F32, tag="ot",
                                        name=f"ot{cc}_{m}_{so}")
                        nc.vector.tensor_scalar(ot[:], ps[:], 1.0 / W_SCALE,
                                                bt[m][:], ALU.mult, ALU.add)
                        nc.sync.dma_start(
                            o_d[m * 128:(m + 1) * 128, off + so: off + so + sn],
                            ot[:])

    nc.compile()
    return nc


_NC_CACHE = {}


def _get_nc(S, niter=1):
    key = (tuple(S), niter)
    if key not in _NC_CACHE:
        _NC_CACHE[key] = _build_nc(S, niter)
    return _NC_CACHE[key]


# ---------------- host wrapper ----------------

def _prepare(x, logits, cheby_coeffs, base_weight, gating_weights, arange):
    x = np.asarray(x, dtype=np.float32)
    logits = np.asarray(logits, dtype=np.float32)
    cheby_coeffs = np.asarray(cheby_coeffs, dtype=np.float32)
    base_weight = np.asarray(base_weight, dtype=np.float32)
    gating_weights = np.asarray(gating_weights, dtype=np.float32)
    arange = np.asarray(arange)

    # top-k routing (host; 8 numbers). Matches jax.lax.top_k ordering.
    order = np.argsort(-logits, kind="stable")[:TOPK]
    topk_vals = 1.0 / (1.0 + np.exp(-logits[order].astype(np.float64)))
    gate = gating_weights.astype(np.float64).copy()
    sel = order + BASE_DEGREES + 1
    gate[sel] = topk_vals
    S = sorted(int(v) for v in sel)

    low = sorted(int(v) for v in arange)   # normally [0..8]
    ops, cheb_cols, X = _solve_basis(S, low)
    _, vec = _recipe(S)
    leaves = _leaf_cols(ops, cheb_cols)
    bf_cheb = [c for c in cheb_cols if c not in leaves]

    # true (f64) weight blocks per column via the basis solve
    G = {n: gate[n] * cheby_coeffs[:, :, n].astype(np.float64) for n in set(low) | set(S)}
    Wtrue = {"silu": base_weight.T.astype(np.float64)}
    bias = np.zeros(O_DIM, dtype=np.float64)
    for j, cn in enumerate(cheb_cols):
        Wb = np.zeros((I_DIM, O_DIM), dtype=np.float64)
        for n, sol in X.items():
            coef = sol[1 + j]
            if coef != 0.0 and n in G:
                Wb += coef * G[n]
        Wtrue[cn] = Wb
    for n, sol in X.items():
        if sol[0] != 0.0 and n in G:
            bias += sol[0] * G[n].sum(axis=0)

    # fp8-quantize leaf blocks (scaled); error-feedback: fold the recoverable
    # Chebyshev content of the quantization residual into the bf16 blocks.
    W8q = {}
    Wadd = {cn: 0.0 for cn in bf_cheb}
    if leaves:
        Alow = np.zeros((40, 1 + len(bf_cheb)))
        Alow[0, 0] = 1.0
        for j, cn in enumerate(bf_cheb):
            Alow[:, 1 + j] = vec[cn]
        comp = {}
        for cn in leaves:
            q = (Wtrue[cn] * W_SCALE).astype(np.float32).astype(F8NP)
            W8q[cn] = q
            dW = Wtrue[cn] - q.astype(np.float64) / W_SCALE
            v = vec[cn]
            for n in np.nonzero(np.abs(v) > 1e-9)[0]:
                if n == 0:
                    bias += v[0] * dW.sum(axis=0)
                else:
                    comp[int(n)] = comp.get(int(n), 0.0) + v[n] * dW
        for n, V in comp.items():
            sol, *_ = np.linalg.lstsq(Alow, _e(n), rcond=None)
            if np.abs(Alow @ sol - _e(n)).max() > 1e-9:
                continue  # degree only carried by an fp8 column; leave as is
            bias += sol[0] * V.sum(axis=0)
            for j, cn in enumerate(bf_cheb):
                if sol[1 + j] != 0.0:
                    Wadd[cn] = Wadd[cn] + sol[1 + j] * V

    # device layouts (all weights scaled by W_SCALE; evac descales)
    bf_blocks = bf_cheb + ["silu"]
    Wsb = np.empty((128, 2 * len(bf_blocks), O_DIM), dtype=np.float32)
    for j, cn in enumerate(bf_blocks):
        Wf = ((Wtrue[cn] + (Wadd.get(cn, 0.0))) * W_SCALE).astype(np.float32)
        Wsb[:, 2 * j + 0, :] = Wf[0:128, :]
        Wsb[:, 2 * j + 1, :] = Wf[128:256, :]
    w_np = Wsb.reshape(128, 2 * len(bf_blocks) * O_DIM).astype(BF16)
    w8_np = None
    if leaves:
        W8sb = np.empty((128, 2 * len(leaves), O_DIM), dtype=F8NP)
        for b8, cn in enumerate(leaves):
            W8sb[:, 2 * b8 + 0, :] = W8q[cn][0:128, :]
            W8sb[:, 2 * b8 + 1, :] = W8q[cn][128:256, :]
        w8_np = W8sb.reshape(128, 2 * len(leaves) * O_DIM)
    bias_np = bias.astype(np.float32).reshape(O_DIM, 1)
    return S, w_np, w8_np, bias_np


def _make_xt(xl):
    """xt[p, 2*off + h*bc + bb] = xl[off+bb, 128*h+p] for each chunk (off, bc)."""
    out = np.empty((128, 2 * B_LOC), dtype=np.float32)
    off = 0
    for bc in CHUNK_SIZES:
        blk = xl[off:off + bc, :].reshape(bc, 2, 128).transpose(2, 1, 0)
        out[:, 2 * off: 2 * (off + bc)] = blk.reshape(128, 2 * bc)
        off += bc
    return out


def _make_in_maps(x, w_np, w8_np, bias_np):
    in_maps = []
    for c in range(N_CORES):
        m = {"xt": _make_xt(x[c * B_LOC:(c + 1) * B_LOC, :]),
             "w": w_np, "bias": bias_np}
        if w8_np is not None:
            m["w8"] = w8_np
        in_maps.append(m)
    return in_maps


def kernel(x, t, logits, cheby_coeffs, base_weight, gating_weights, arange):
    x = np.asarray(x, dtype=np.float32)
    S, w_np, w8_np, bias_np = _prepare(x, logits, cheby_coeffs, base_weight,
                                       gating_weights, arange)
    nc = _get_nc(S)
    in_maps = _make_in_maps(x, w_np, w8_np, bias_np)
    res = run_bass_kernel_spmd(nc, in_maps, core_ids=list(range(N_CORES)))
    y = np.empty((BATCH, O_DIM), dtype=np.float32)
    for c in range(N_CORES):
        y[c * B_LOC:(c + 1) * B_LOC, :] = res.results[c]["out"].T
    return y


# revision 37
# speedup vs baseline: 1.4450x; 1.0581x over previous
"""AChebyKANLinear forward on 8 TRN2 NeuronCores (data-parallel over batch).

y = silu(x) @ W_base^T + einsum('bid,iod->bo', cos(n_d * arccos(tanh x)), gated_coeffs)

Key identities used:
  cos(n*arccos(c)) = T_n(c)  (Chebyshev), c = tanh(x)
  -> no trig needed on device. Device computes 13 "columns" per feature:
     silu(x), and 12 cheap polynomials of c whose exact Chebyshev-basis
     expansion is tracked symbolically on the host; the host solves a small
     linear system to fold the change of basis into the matmul weights.
  Even T_2m come from ACT Square ops (T_2m+1 = 2*T_m^2), odd ones from single
  fused DVE scalar_tensor_tensor ops. All columns bf16; one big
  [4096, 3328] @ [3328, 256] GEMM per core on TensorE (fp32 PSUM accum).

Top-k routing over the 8 logits is computed on the host (it is 8 numbers);
the 4 selected high degrees are baked into the compiled graph.
"""

import numpy as np
import ml_dtypes
from contextlib import ExitStack

import concourse.bass as bass
import concourse.tile as tile
from concourse import bacc, mybir
from concourse.bass_utils import run_bass_kernel_spmd

BF16 = ml_dtypes.bfloat16

N_CORES = 8
BATCH, I_DIM, O_DIM = 32768, 256, 256
B_LOC = BATCH // N_CORES          # 4096
# graduated batch chunks: small first chunks shorten the pipeline fill before
# TensorE has all 13 columns of chunk 0; steady state runs at 1024.
CHUNK_SIZES = [256, 256, 512, 1024, 1024, 1024]
assert sum(CHUNK_SIZES) == B_LOC
BC_MAX = max(CHUNK_SIZES)
DEGREE = 16
BASE_DEGREES = 8
TOPK = 4

SQ2 = float(np.sqrt(2.0))

A = mybir.ActivationFunctionType
ALU = mybir.AluOpType
F32 = mybir.dt.float32
DBF16 = mybir.dt.bfloat16


# ---------------- symbolic Chebyshev algebra (host, exact) ----------------

def _chmul(a, b):
    out = np.zeros(40)
    nz_a = np.nonzero(a)[0]
    nz_b = np.nonzero(b)[0]
    for i in nz_a:
        for j in nz_b:
            p = a[i] * b[j] * 0.5
            out[i + j] += p
            out[abs(i - j)] += p
    return out


def _e(n):
    v = np.zeros(40)
    v[n] = 1.0
    return v


def _recipe(S):
    """Build the per-chunk op recipe and each column's Chebyshev expansion.

    Returns (ops, colvec) where ops is a list of
      ('act', name, src, func, scale, bias) or
      ('stt', name, in0, scalar, op0, in1, op1) or
      ('tt',  name, in0, in1, op)
    and colvec maps tile name -> length-40 Chebyshev coefficient vector.
    """
    ops = []
    vec = {}

    def act(name, src, func, scale=1.0, bias=0.0):
        ops.append(("act", name, src, func, float(scale), float(bias)))
        if func == A.Square:
            aff = vec[src] * scale
            aff[0] += bias
            vec[name] = _chmul(aff, aff)
        elif func == A.Tanh:
            vec[name] = _e(1)
        else:  # Silu: not a Chebyshev column
            vec[name] = None

    def stt(name, in0, scalar, op0, in1, op1):
        ops.append(("stt", name, in0, float(scalar), op0, in1, op1))
        a = vec[in0].copy()
        if op0 == ALU.add:
            a[0] += scalar
        elif op0 == ALU.mult:
            a = a * scalar
        else:
            raise ValueError(op0)
        b = vec[in1]
        if op1 == ALU.mult:
            vec[name] = _chmul(a, b)
        elif op1 == ALU.subtract:
            vec[name] = a - b
        elif op1 == ALU.add:
            vec[name] = a + b
        else:
            raise ValueError(op1)

    def tt(name, in0, in1, op):
        ops.append(("tt", name, in0, in1, op))
        if op == ALU.subtract:
            vec[name] = vec[in0] - vec[in1]
        elif op == ALU.add:
            vec[name] = vec[in0] + vec[in1]
        elif op == ALU.mult:
            vec[name] = _chmul(vec[in0], vec[in1])
        else:
            raise ValueError(op)

    # c1 first: it unblocks the whole Square chain and every DVE op; silu is
    # only consumed by the last K-blocks, so it is produced late.
    act("c1", "x", A.Tanh)
    act("c2", "c1", A.Square, SQ2)            # T2 + 1
    stt("c3", "c2", -1.5, ALU.add, "c1", ALU.mult)   # (c2-1.5)*c1 = T3/2
    act("c4", "c2", A.Square, SQ2, -SQ2)      # T4 + 1
    stt("c5", "c4", -1.0, ALU.add, "c1", ALU.mult)   # T4*T1 = (T5+T3)/2
    stt("c6", "c3", 2.0, ALU.mult, "c3", ALU.mult)   # 2*c3^2 = (T6+1)/2  (DVE)
    stt("c7", "c4", -1.0, ALU.add, "c3", ALU.mult)   # T4*T3/2 = (T7+T1)/4
    act("c8", "c4", A.Square, SQ2, -SQ2)      # T8 + 1
    act("silu", "x", A.Silu)
    for n in S:
        if n == 9:
            stt("c9", "c8", -1.0, ALU.add, "c1", ALU.mult)    # T8*T1
        elif n == 10:
            act("c10", "c5", A.Square, SQ2)                    # 2*c5^2
        elif n == 11:
            stt("c11", "c8", -1.0, ALU.add, "c3", ALU.mult)   # T8*T3/2
        elif n == 12:
            act("c12", "c6", A.Square, 2.0 * SQ2, -SQ2)        # 2*(2*c6-1)^2 = T12+1
        elif n == 13:
            tt("d53", "c5", "c3", ALU.subtract)                # T5/2
            stt("c13", "c8", -1.0, ALU.add, "d53", ALU.mult)  # T8*T5/2
        elif n == 14:
            stt("c14", "c8", -1.0, ALU.add, "c6", ALU.mult)   # T8*(T6+1)/2
        elif n == 15:
            stt("t7p", "c7", 4.0, ALU.mult, "c1", ALU.subtract)  # T7
            stt("c15", "c8", -1.0, ALU.add, "t7p", ALU.mult)     # T8*T7
        elif n == 16:
            act("c16", "c8", A.Square, SQ2, -SQ2)              # T16+1
        else:
            raise ValueError(n)
    return ops, vec


def _solve_basis(S, low_degrees):
    """Solve for X s.t. sum_col X[col,n]*colvec[col] == e_n for each needed n.

    Columns: 'bias' (the constant 1) + the 12 device Chebyshev columns.
    Returns (ops, matmul_cols, X) with X keyed [col][n].
    """
    ops, vec = _recipe(S)
    cheb_cols = ["c1", "c2", "c3", "c4", "c5", "c6", "c7", "c8"] + [f"c{n}" for n in S]
    needed = sorted(set(int(n) for n in low_degrees) | set(S))
    Amat = np.zeros((40, 1 + len(cheb_cols)))
    Amat[0, 0] = 1.0  # bias column = T_0
    for j, cn in enumerate(cheb_cols):
        Amat[:, 1 + j] = vec[cn]
    X = {}
    for n in needed:
        sol, res, rank, _ = np.linalg.lstsq(Amat, _e(n), rcond=None)
        err = np.abs(Amat @ sol - _e(n)).max()
        assert err < 1e-9, f"basis solve failed for degree {n}: {err}"
        X[n] = sol  # [1+len(cheb_cols)]
    return ops, cheb_cols, X


# ---------------- device graph ----------------

# fp8 "leaf" columns: columns no chain op reads can be written float8_e4m3
# directly and contracted with DoubleRow matmuls (2 k-rows per instruction).
# All weights are scaled by W_SCALE on host (so fp8 weights avoid subnormals);
# the PSUM is descaled during the bias-add evacuation.
# Measured on HW: DoubleRow groups are 1.4x faster in an isolated microbench,
# but in this kernel the 256-col non-FWL LDWEIGHTS exposure cancels the gain
# (A/B medians 94us fp8 vs 91us bf16) while costing rel-err 8e-3 vs 4.2e-3.
# Kept implemented but disabled.
FP8_LEAVES = False
R_BUFS = 2
T_BUFS = 2
W_SCALE = 4096.0
DF8 = mybir.dt.float8e4
F8NP = mybir.dt.np(mybir.dt.float8e4)


def _leaf_cols(ops, cheb_cols):
    if not FP8_LEAVES:
        return []
    read = set()
    for op in ops:
        if op[0] == "act":
            read.add(op[2])
        elif op[0] == "stt":
            read.add(op[2])
            read.add(op[5])
        else:
            read.add(op[2])
            read.add(op[3])
    return [c for c in cheb_cols if c not in read]


def _build_nc(S, niter=1, ablate=None):
    # ablate: None (normal), "producers" (memset columns once; PE/DMA path only)
    ops, cheb_cols, _ = _solve_basis(S, range(BASE_DEGREES + 1))
    leaves = _leaf_cols(ops, cheb_cols)
    bf_blocks = [c for c in cheb_cols if c not in leaves] + ["silu"]
    n_kk = 2 * len(bf_blocks)              # bf16 k-tiles of 128
    n8 = len(leaves)                       # fp8 DoubleRow blocks (256 k-rows each)

    nc = bacc.Bacc("TRN2", target_bir_lowering=False, debug=False,
                   num_devices=N_CORES)
    # register const APs for the activation bias values we use (only 0.0/1.0
    # are pre-registered); mirrors Bass.__init__'s register_const_ap.
    bias_consts = sorted({op[5] for op in ops if op[0] == "act"} - {0.0})
    for v in bias_consts:
        t_c = nc.alloc_sbuf_tensor(f"const-f32-{v}", [128, 1], F32)
        nc.gpsimd.memset(t_c.ap(), v)
        nc.const_aps.aps[(F32, v)] = t_c.ap()
    if bias_consts:
        nc.all_engine_barrier()
    x_d = nc.dram_tensor("xt", [128, 2 * B_LOC], F32, kind="ExternalInput").ap()
    w_d = nc.dram_tensor("w", [128, n_kk * O_DIM], DBF16, kind="ExternalInput").ap()
    if n8:
        w8_d = nc.dram_tensor("w8", [128, n8 * 2 * O_DIM], DF8,
                              kind="ExternalInput").ap()
    b_d = nc.dram_tensor("bias", [O_DIM, 1], F32, kind="ExternalInput").ap()
    o_d = nc.dram_tensor("out", [O_DIM, B_LOC], F32, kind="ExternalOutput").ap()

    with tile.TileContext(nc) as tc, ExitStack() as ctx:
        cpool = ctx.enter_context(tc.tile_pool(name="const", bufs=1))
        xpool = ctx.enter_context(tc.tile_pool(name="x", bufs=3))
        rpool = ctx.enter_context(tc.tile_pool(name="r", bufs=R_BUFS))
        tpool = ctx.enter_context(tc.tile_pool(name="tmp", bufs=T_BUFS))
        opool = ctx.enter_context(tc.tile_pool(name="o", bufs=4))
        pspool = ctx.enter_context(tc.tile_pool(name="ps", bufs=4, space="PSUM"))

        wt = cpool.tile([128, n_kk * O_DIM], DBF16)
        nc.sync.dma_start(wt[:], w_d[:])
        if n8:
            w8t = cpool.tile([128, n8 * 2 * O_DIM], DF8)
            nc.sync.dma_start(w8t[:], w8_d[:])
            w8v = w8t[:].rearrange("p (b r o) -> p b r o", r=2, o=O_DIM)
        bt = []
        for m in range(2):
            b_tile = cpool.tile([128, 1], F32, tag=f"bias{m}")
            nc.sync.dma_start(b_tile[:], b_d[m * 128:(m + 1) * 128, :])
            bt.append(b_tile)

        chunks = []
        off = 0
        for bc in CHUNK_SIZES:
            chunks.append((off, bc))
            off += bc
        const_tiles = None
        if ablate == "producers":
            const_tiles = {}
            for nm in (["silu"] + cheb_cols):
                ct = cpool.tile([128, 2 * BC_MAX],
                                DF8 if nm in leaves else DBF16,
                                tag=f"ab_{nm}", name=f"ab_{nm}")
                nc.vector.memset(ct[:], 0.5)
                const_tiles[nm] = ct
        for it in range(niter):
            for ci, (off, bc) in enumerate(chunks):
                cc = f"{it}_{ci}"
                xt = xpool.tile([128, 2 * bc], F32, tag="xt", name=f"xt{cc}")
                nc.sync.dma_start(xt[:], x_d[:, 2 * off: 2 * (off + bc)])

                tiles = {"x": xt}
                if ablate == "producers":
                    tiles.update(const_tiles)
                else:
                    for op in ops:
                        kind, name = op[0], op[1]
                        is_col = (name in bf_blocks) or (name in leaves)
                        pool = rpool if is_col else tpool
                        dt_col = DF8 if name in leaves else DBF16
                        t = pool.tile([128, 2 * bc], dt_col, tag=name,
                                      name=f"{name}_{cc}")
                        if kind == "act":
                            _, _, src, func, scale, bias_v = op
                            nc.scalar.activation(t[:], tiles[src][:], func,
                                                 bias=bias_v, scale=scale)
                        elif kind == "stt":
                            _, _, in0, scalar, op0, in1, op1 = op
                            nc.vector.scalar_tensor_tensor(t[:], tiles[in0][:], scalar,
                                                           tiles[in1][:], op0, op1)
                        else:  # tt
                            _, _, in0, in1, alu = op
                            nc.vector.tensor_tensor(t[:], tiles[in0][:],
                                                    tiles[in1][:], alu)
                        tiles[name] = t

                nsubs = [(s, min(512, bc - s)) for s in range(0, bc, 512)]
                n_mm = n_kk + n8
                for m in range(2):
                    # one PSUM tile spans all n-subtiles (up to 2 banks); each
                    # 512-wide accumulation group writes its own bank, and a
                    # single evac + output DMA covers the whole chunk.
                    ps = pspool.tile([128, bc], F32, tag="ps",
                                     name=f"ps{cc}_{m}")
                    for so, sn in nsubs:
                        mi = 0
                        for kk in range(n_kk):
                            j, h = kk // 2, kk % 2
                            rt = tiles[bf_blocks[j]]
                            nc.tensor.matmul(
                                ps[:, so:so + sn],
                                wt[:, kk * O_DIM + m * 128: kk * O_DIM + (m + 1) * 128],
                                rt[:, h * bc + so: h * bc + so + sn],
                                start=(mi == 0), stop=(mi == n_mm - 1),
                            )
                            mi += 1
                        for b8, lf in enumerate(leaves):
                            rt = tiles[lf]
                            rhs = rt[:].rearrange("p (r n) -> p r n", r=2)[:, :, so:so + sn]
                            nc.tensor.matmul(
                                ps[:, so:so + sn],
                                w8v[:, b8, :, m * 128:(m + 1) * 128],
                                rhs,
                                start=(mi == 0), stop=(mi == n_mm - 1),
                                perf_mode=mybir.MatmulPerfMode.DoubleRow,
                            )
                            mi += 1
                    ot = opool.tile([128, bc], F32, tag="ot",
                                    name=f"ot{cc}_{m}")
                    nc.vector.tensor_scalar(ot[:], ps[:], 1.0 / W_SCALE,
                                            bt[m][:], ALU.mult, ALU.add)
                    nc.sync.dma_start(
                        o_d[m * 128:(m + 1) * 128, off: off + bc],
                        ot[:])

    nc.compile()
    return nc


_NC_CACHE = {}


def _get_nc(S, niter=1):
    key = (tuple(S), niter)
    if key not in _NC_CACHE:
        _NC_CACHE[key] = _build_nc(S, niter)
    return _NC_CACHE[key]


# ---------------- host wrapper ----------------

def _prepare(x, logits, cheby_coeffs, base_weight, gating_weights, arange):
    x = np.asarray(x, dtype=np.float32)
    logits = np.asarray(logits, dtype=np.float32)
    cheby_coeffs = np.asarray(cheby_coeffs, dtype=np.float32)
    base_weight = np.asarray(base_weight, dtype=np.float32)
    gating_weights = np.asarray(gating_weights, dtype=np.float32)
    arange = np.asarray(arange)

    # top-k routing (host; 8 numbers). Matches jax.lax.top_k ordering.
    order = np.argsort(-logits, kind="stable")[:TOPK]
    topk_vals = 1.0 / (1.0 + np.exp(-logits[order].astype(np.float64)))
    gate = gating_weights.astype(np.float64).copy()
    sel = order + BASE_DEGREES + 1
    gate[sel] = topk_vals
    S = sorted(int(v) for v in sel)

    low = sorted(int(v) for v in arange)   # normally [0..8]
    ops, cheb_cols, X = _solve_basis(S, low)
    _, vec = _recipe(S)
    leaves = _leaf_cols(ops, cheb_cols)
    bf_cheb = [c for c in cheb_cols if c not in leaves]

    # true (f64) weight blocks per column via the basis solve
    G = {n: gate[n] * cheby_coeffs[:, :, n].astype(np.float64) for n in set(low) | set(S)}
    Wtrue = {"silu": base_weight.T.astype(np.float64)}
    bias = np.zeros(O_DIM, dtype=np.float64)
    for j, cn in enumerate(cheb_cols):
        Wb = np.zeros((I_DIM, O_DIM), dtype=np.float64)
        for n, sol in X.items():
            coef = sol[1 + j]
            if coef != 0.0 and n in G:
                Wb += coef * G[n]
        Wtrue[cn] = Wb
    for n, sol in X.items():
        if sol[0] != 0.0 and n in G:
            bias += sol[0] * G[n].sum(axis=0)

    # fp8-quantize leaf blocks (scaled); error-feedback: fold the recoverable
    # Chebyshev content of the quantization residual into the bf16 blocks.
    W8q = {}
    Wadd = {cn: 0.0 for cn in bf_cheb}
    if leaves:
        Alow = np.zeros((40, 1 + len(bf_cheb)))
        Alow[0, 0] = 1.0
        for j, cn in enumerate(bf_cheb):
            Alow[:, 1 + j] = vec[cn]
        comp = {}
        for cn in leaves:
            q = (Wtrue[cn] * W_SCALE).astype(np.float32).astype(F8NP)
            W8q[cn] = q
            dW = Wtrue[cn] - q.astype(np.float64) / W_SCALE
            v = vec[cn]
            for n in np.nonzero(np.abs(v) > 1e-9)[0]:
                if n == 0:
                    bias += v[0] * dW.sum(axis=0)
                else:
                    comp[int(n)] = comp.get(int(n), 0.0) + v[n] * dW
        for n, V in comp.items():
            sol, *_ = np.linalg.lstsq(Alow, _e(n), rcond=None)
            if np.abs(Alow @ sol - _e(n)).max() > 1e-9:
                continue  # degree only carried by an fp8 column; leave as is
            bias += sol[0] * V.sum(axis=0)
            for j, cn in enumerate(bf_cheb):
                if sol[1 + j] != 0.0:
                    Wadd[cn] = Wadd[cn] + sol[1 + j] * V

    # device layouts (all weights scaled by W_SCALE; evac descales)
    bf_blocks = bf_cheb + ["silu"]
    Wsb = np.empty((128, 2 * len(bf_blocks), O_DIM), dtype=np.float32)
    for j, cn in enumerate(bf_blocks):
        Wf = ((Wtrue[cn] + (Wadd.get(cn, 0.0))) * W_SCALE).astype(np.float32)
        Wsb[:, 2 * j + 0, :] = Wf[0:128, :]
        Wsb[:, 2 * j + 1, :] = Wf[128:256, :]
    w_np = Wsb.reshape(128, 2 * len(bf_blocks) * O_DIM).astype(BF16)
    w8_np = None
    if leaves:
        W8sb = np.empty((128, 2 * len(leaves), O_DIM), dtype=F8NP)
        for b8, cn in enumerate(leaves):
            W8sb[:, 2 * b8 + 0, :] = W8q[cn][0:128, :]
            W8sb[:, 2 * b8 + 1, :] = W8q[cn][128:256, :]
        w8_np = W8sb.reshape(128, 2 * len(leaves) * O_DIM)
    bias_np = bias.astype(np.float32).reshape(O_DIM, 1)
    return S, w_np, w8_np, bias_np


def _make_xt(xl):
    """xt[p, 2*off + h*bc + bb] = xl[off+bb, 128*h+p] for each chunk (off, bc)."""
    out = np.empty((128, 2 * B_LOC), dtype=np.float32)
    off = 0
    for bc in CHUNK_SIZES:
        blk = xl[off:off + bc, :].reshape(bc, 2, 128).transpose(2, 1, 0)
        out[:, 2 * off: 2 * (off + bc)] = blk.reshape(128, 2 * bc)
        off += bc
    return out


def _make_in_maps(x, w_np, w8_np, bias_np):
    in_maps = []
    for c in range(N_CORES):
        m = {"xt": _make_xt(x[c * B_LOC:(c + 1) * B_LOC, :]),
             "w": w_np, "bias": bias_np}
        if w8_np is not None:
            m["w8"] = w8_np
        in_maps.append(m)
    return in_maps


def kernel(x, t, logits, cheby_coeffs, base_weight, gating_weights, arange):
    x = np.asarray(x, dtype=np.float32)
    S, w_np, w8_np, bias_np = _prepare(x, logits, cheby_coeffs, base_weight,
                                       gating_weights, arange)
    nc = _get_nc(S)
    in_maps = _make_in_maps(x, w_np, w8_np, bias_np)
    res = run_bass_kernel_spmd(nc, in_maps, core_ids=list(range(N_CORES)))
    y = np.empty((BATCH, O_DIM), dtype=np.float32)
    for c in range(N_CORES):
        y[c * B_LOC:(c + 1) * B_LOC, :] = res.results[c]["out"].T
    return y
